# revision 10
# baseline (speedup 1.0000x reference)
"""Trainium2 Bass kernel for nn_DSSM_57629871178390 (dual-stream Mamba/DSSM block).

Sharding: d_inner=256 split 8 ways across cores (Dsh=32 channels each). The
selective scan runs on the DVE via tensor_tensor_scan with 128-partition
tiles laid out as (dd=8 d-values x n=16 states); K=3 x G=4 tiles per (b,chunk).

v2 restructure vs baseline:
- softplus(dt) via Exp+Ln (one ACT table with the scan's Exp -> no table loads)
- dt for all 3 scan routes fused into one [96,Tc] matmul+ACT chain
- ndt replication via PE matmul from a [8,12*Tc] DRAM-bounced staging tile
- b_t muls on the Pool engine (GpSimd), hc muls mostly on DVE
- inter stored interleaved in DRAM (no per-load de-interleave copies)
- per-batch AllReduce of x_dbl, overlapped with the other batch's scan
- ReduceScatter split per stream (ms/pan) to overlap with out_proj
- scan carry chained via previous h tile's last column (no copy ops)

Self-contained: hardcodes all shapes; imports numpy + concourse (available
at /opt/trn_rl_repo inside the container).
"""

import sys
from dataclasses import dataclass

import numpy as np

if "/opt/trn_rl_repo" not in sys.path:
    sys.path.insert(0, "/opt/trn_rl_repo")

import ml_dtypes  # noqa: E402

import concourse.bass as bass  # noqa: E402
import concourse.bacc as bacc  # noqa: E402
import concourse.tile as tile  # noqa: E402
import concourse.mybir as mybir  # noqa: E402
from concourse import bass_utils  # noqa: E402

F32 = mybir.dt.float32
BF16 = mybir.dt.bfloat16
AF = mybir.ActivationFunctionType
OP = mybir.AluOpType
NPBF16 = ml_dtypes.bfloat16


@dataclass(frozen=True)
class Cfg:
    B: int = 2
    C: int = 128
    H: int = 64
    W: int = 64
    D: int = 256          # d_inner
    N: int = 16           # d_state
    R: int = 8            # dt_rank
    K: int = 3
    n_cores: int = 8
    fake_cc: bool = False   # replace collectives with plain DMA (profiling)

    @property
    def L(self):
        return self.H * self.W

    @property
    def L2(self):
        return 2 * self.L

    @property
    def Dsh(self):
        return self.D // self.n_cores

    @property
    def DG(self):
        return 128 // self.N       # d-values per 128-partition tile (8)

    @property
    def G(self):
        return self.Dsh // self.DG  # d-groups per core (4)

    @property
    def Tc(self):
        return 1024                 # scan chunk

    @property
    def TOK(self):
        return min(512, self.L)     # in_proj token chunk

    @property
    def CR(self):
        return max(1, min(512 // self.W, self.H))  # conv rows per chunk

    @property
    def TcL(self):
        return min(512, self.L)     # LN/out_proj chunk


CFG = Cfg()
MMF = 512  # max moving free dim per matmul


# ---------------------------------------------------------------------------
# Host-side preparation of per-core input maps
# ---------------------------------------------------------------------------

def host_prep(cfg: Cfg, inputs: dict) -> list:
    B, C, H, W = cfg.B, cfg.C, cfg.H, cfg.W
    D, N, R, K = cfg.D, cfg.N, cfg.R, cfg.K
    Dsh, G, DG, L = cfg.Dsh, cfg.G, cfg.DG, cfg.L

    ms = np.asarray(inputs["ms"], np.float32).reshape(B, C, L)
    pan = np.asarray(inputs["pan"], np.float32).reshape(B, C, L)
    w_ms = np.asarray(inputs["in_proj_ms_w"], np.float32)
    w_pan = np.asarray(inputs["in_proj_pan_w"], np.float32)
    cw_ms = np.asarray(inputs["conv_ms_w"], np.float32)
    cb_ms = np.asarray(inputs["conv_ms_b"], np.float32)
    cw_pan = np.asarray(inputs["conv_pan_w"], np.float32)
    cb_pan = np.asarray(inputs["conv_pan_b"], np.float32)
    xpw = np.asarray(inputs["x_proj_weight"], np.float32)
    dtw = np.asarray(inputs["dt_projs_weight"], np.float32)
    dtb = np.asarray(inputs["dt_projs_bias"], np.float32)
    A_logs = np.asarray(inputs["A_logs"], np.float32)
    Ds = np.asarray(inputs["Ds"], np.float32)
    lnw_v = np.asarray(inputs["out_norm_vis_w"], np.float32)
    lnb_v = np.asarray(inputs["out_norm_vis_b"], np.float32)
    lnw_i = np.asarray(inputs["out_norm_inf_w"], np.float32)
    lnb_i = np.asarray(inputs["out_norm_inf_b"], np.float32)
    opw_ms = np.asarray(inputs["out_proj_ms_w"], np.float32)
    opw_pan = np.asarray(inputs["out_proj_pan_w"], np.float32)

    A = np.exp(A_logs).reshape(K, D, N)
    Dsum = Ds.reshape(K, D).sum(0)

    # partition layout of scan tiles: p = dd*N + n
    red = np.zeros((128, G * Dsh), NPBF16)
    for g in range(G):
        for p in range(128):
            red[p, g * Dsh + DG * g + (p // N)] = 1.0

    selst_h = np.zeros((128, 2, 4 * B), NPBF16)
    for pi in range(128):
        selst_h[pi, 0, pi // Dsh] = 1.0          # S1 rows
        selst_h[pi, 1, 2 * B + pi // Dsh] = 1.0  # S2 rows
    seldd = np.zeros((DG, 128), NPBF16)
    for pi in range(128):
        seldd[pi // N, pi] = 1.0

    shared = {
        "seldd": seldd,
        "selst": selst_h.reshape(128, 2 * 4 * B),
        "msf": ms.astype(NPBF16),
        "panf": pan.astype(NPBF16),
        "red": red,
    }

    maps = []
    for c in range(cfg.n_cores):
        dsl = slice(Dsh * c, Dsh * (c + 1))
        m = dict(shared)
        # cols 0:Dsh = z rows (W[D+dsl]), cols Dsh:2Dsh = x rows (W[dsl])
        m["w_in_ms"] = np.ascontiguousarray(
            np.concatenate([w_ms[D + Dsh * c: D + Dsh * (c + 1)].T,
                            w_ms[dsl].T], 1)).astype(NPBF16)
        m["w_in_pan"] = np.ascontiguousarray(
            np.concatenate([w_pan[D + Dsh * c: D + Dsh * (c + 1)].T,
                            w_pan[dsl].T], 1)).astype(NPBF16)
        cd_ms = np.zeros((Dsh, 9 * Dsh), NPBF16)
        cd_pan = np.zeros((Dsh, 9 * Dsh), NPBF16)
        for t in range(9):
            ky, kx = t // 3, t % 3
            for i in range(Dsh):
                cd_ms[i, t * Dsh + i] = cw_ms[Dsh * c + i, 0, ky, kx]
                cd_pan[i, t * Dsh + i] = cw_pan[Dsh * c + i, 0, ky, kx]
        m["conv_d_ms"] = cd_ms
        m["conv_d_pan"] = cd_pan
        m["conv_b_ms"] = cb_ms[dsl].reshape(Dsh, 1).astype(np.float32)
        m["conv_b_pan"] = cb_pan[dsl].reshape(Dsh, 1).astype(np.float32)
        m["xproj_T"] = np.ascontiguousarray(
            xpw[:, :, dsl].transpose(2, 0, 1).reshape(Dsh, K * (R + 2 * N))
        ).astype(NPBF16)
        # fused dt projection: out partition = k*Dsh + dl
        dtw02 = np.zeros((2 * R, 3 * Dsh), NPBF16)
        dtw1 = np.zeros((R, 3 * Dsh), NPBF16)
        for r in range(R):
            for dl in range(Dsh):
                dtw02[r, dl] = dtw[0, Dsh * c + dl, r]
                dtw02[R + r, 2 * Dsh + dl] = dtw[2, Dsh * c + dl, r]
                dtw1[r, Dsh + dl] = dtw[1, Dsh * c + dl, r]
        m["dtw02"] = dtw02
        m["dtw1"] = dtw1
        m["dtb96"] = np.ascontiguousarray(
            dtb[:, dsl].reshape(K * Dsh, 1)).astype(np.float32)
        acol = np.zeros((128, K * G), np.float32)
        for k in range(K):
            for g in range(G):
                for p in range(128):
                    dd, n = p // N, p % N
                    acol[p, k * G + g] = -A[k, Dsh * c + DG * g + dd, n]
        m["A_col"] = acol
        Ddiag = np.zeros((Dsh, Dsh), NPBF16)
        for i in range(Dsh):
            Ddiag[i, i] = Dsum[Dsh * c + i]
        m["Ddiag"] = Ddiag
        lw4 = np.zeros((128, 2), np.float32)
        for bs in range(2 * B):
            sle = bs % 2
            lw4[bs * Dsh:(bs + 1) * Dsh, 0] = (lnw_v if sle == 0 else lnw_i)[dsl]
            lw4[bs * Dsh:(bs + 1) * Dsh, 1] = (lnb_v if sle == 0 else lnb_i)[dsl]
        m["lnwb4"] = lw4
        o4 = np.zeros((128, C), NPBF16)
        for bs in range(2 * B):
            sle = bs % 2
            o4[bs * Dsh:(bs + 1) * Dsh] = (
                (opw_ms if sle == 0 else opw_pan)[:, dsl].T).astype(NPBF16)
        m["opw4"] = o4
        m["opw"] = np.ascontiguousarray(
            np.concatenate([opw_ms[:, dsl].T, opw_pan[:, dsl].T], 1)
        ).astype(NPBF16)  # (Dsh, 2C)
        maps.append(m)
    return maps


# ---------------------------------------------------------------------------
# AP helpers
# ---------------------------------------------------------------------------

def rep_sbuf(ap2d, n_inner):
    """SBUF [P, F] slice -> src AP for dest [P*n_inner, F] with each source
    partition repeated n_inner times (dest p = i*n_inner + j reads src i)."""
    pairs = [list(x) for x in ap2d.ap]
    assert len(pairs) == 2
    return bass.AP(tensor=ap2d.tensor, offset=ap2d.offset,
                   ap=[pairs[0], [0, n_inner], pairs[1]])


def rep_dram(tensor_ap, offset, row_stride, n_rows, n_rep, fstep, fcount):
    """DRAM source AP replicating a [n_rows, fcount] block so that dest
    partition p = j*n_rows + r reads row r (rep index j outer)."""
    return bass.AP(tensor=tensor_ap.tensor, offset=tensor_ap.offset + offset,
                   ap=[[0, n_rep], [row_stride, n_rows], [fstep, fcount]])


def dram_ap(tensor_ap, offset, pairs):
    return bass.AP(tensor=tensor_ap.tensor, offset=tensor_ap.offset + offset,
                   ap=pairs)


def mm(nc, out_ps, lhsT, rhs, start, stop, maxf=MMF):
    """Matmul with moving-free-dim splitting. out/rhs 2D [P, F]."""
    F = rhs.shape[-1]
    if F <= maxf:
        nc.tensor.matmul(out_ps, lhsT, rhs, start=start, stop=stop,
                         skip_group_check=True)
        return
    assert F % maxf == 0
    for i in range(F // maxf):
        nc.tensor.matmul(out_ps[:, i * maxf:(i + 1) * maxf], lhsT,
                         rhs[:, i * maxf:(i + 1) * maxf], start=start, stop=stop,
                         skip_group_check=True)


# ---------------------------------------------------------------------------
# Bass program builder
# ---------------------------------------------------------------------------

def build_nc(cfg: Cfg):
    B, C, H, W = cfg.B, cfg.C, cfg.H, cfg.W
    D, N, R, K = cfg.D, cfg.N, cfg.R, cfg.K
    Dsh, G, DG = cfg.Dsh, cfg.G, cfg.DG
    L, L2, Tc = cfg.L, cfg.L2, cfg.Tc
    TOK, CRW, TcL = cfg.TOK, cfg.CR, cfg.TcL
    NCH = L2 // Tc               # scan chunks per batch (16)
    PC = 1024                    # x_dbl phase chunk
    XD = R + 2 * N               # 40
    Hp, Wp = H + 2, W + 2
    groups = [list(range(cfg.n_cores))]
    nLch = L // TcL
    SC = 2 * B * L // 64         # stats cols for [64, SC] view
    HMM = 512

    nc = bacc.Bacc("TRN2", target_bir_lowering=False, debug=False,
                   enable_asserts=False, num_devices=cfg.n_cores)

    t = {}

    def inp(name, shape, dt):
        t[name] = nc.dram_tensor(name, shape, dt, kind="ExternalInput").ap()

    inp("msf", [B, C, L], BF16)
    inp("panf", [B, C, L], BF16)
    inp("w_in_ms", [C, 2 * Dsh], BF16)    # cols 0:Dsh z-rows, Dsh:2Dsh x-rows
    inp("w_in_pan", [C, 2 * Dsh], BF16)
    inp("conv_d_ms", [Dsh, 9 * Dsh], BF16)
    inp("conv_d_pan", [Dsh, 9 * Dsh], BF16)
    inp("conv_b_ms", [Dsh, 1], F32)
    inp("conv_b_pan", [Dsh, 1], F32)
    inp("xproj_T", [Dsh, K * XD], BF16)
    inp("dtw02", [2 * R, K * Dsh], BF16)
    inp("dtw1", [R, K * Dsh], BF16)
    inp("dtb96", [K * Dsh, 1], F32)
    inp("A_col", [128, K * G], F32)       # NEGATIVE -exp(A_logs)
    inp("Ddiag", [Dsh, Dsh], BF16)
    inp("red", [128, G * Dsh], BF16)
    inp("seldd", [DG, 128], BF16)
    inp("selst", [128, 2 * 4 * B], BF16)
    inp("lnwb4", [128, 2], F32)
    inp("opw4", [128, C], BF16)
    inp("opw", [Dsh, 2 * C], BF16)

    flat_out = 2 * B * C * L
    chunk_o = flat_out // cfg.n_cores
    out_chunk = nc.dram_tensor("out_chunk", [chunk_o], F32,
                               kind="ExternalOutput").ap()

    shsp = "Shared" if cfg.n_cores > 4 else "Local"
    inter_dram = nc.dram_tensor("inter_dram", [Dsh, B, L2], BF16,
                                kind="Internal").ap()
    zs_dram = nc.dram_tensor("zs_dram", [Dsh, 2 * B, L], BF16,
                             kind="Internal").ap()
    xdbl_part = nc.dram_tensor("xdbl_part", [B, K * XD, L2], BF16,
                               kind="Internal").ap()
    xdbl_full = nc.dram_tensor("xdbl_full", [B, K * XD, L2], BF16,
                               kind="Internal", addr_space=shsp).ap()
    dtg_dram = nc.dram_tensor("dtg_dram", [2, K * Dsh, Tc], BF16,
                              kind="Internal").ap()
    brc_dram = nc.dram_tensor("brc_dram", [2 * N, 2, Tc], BF16,
                              kind="Internal").ap()
    yev_dram = nc.dram_tensor("yev_dram", [Dsh, B, 2, L2], BF16,
                              kind="Internal").ap()
    stats_part = nc.dram_tensor("stats_part", [2, 2 * B, L], F32,
                                kind="Internal").ap()
    stats_full = nc.dram_tensor("stats_full", [2, 2 * B, L], F32,
                                kind="Internal", addr_space=shsp).ap()
    ab_dram = nc.dram_tensor("ab_dram", [2, 2 * B, L], F32,
                             kind="Internal").ap()
    outp_part = nc.dram_tensor("outp_part", [2, B, C, L], F32,
                               kind="Internal").ap()
    outp_rs = nc.dram_tensor("outp_rs", [2 * B * C * L // cfg.n_cores], F32,
                             kind="Internal").ap()

    # persistent SBUF weights
    def sb(name, shape, dt):
        return nc.alloc_sbuf_tensor(name, shape, dt).ap()

    w_in_ms = sb("w_in_ms_s", [C, 2 * Dsh], BF16)
    w_in_pan = sb("w_in_pan_s", [C, 2 * Dsh], BF16)
    convd = sb("convd_s", [2 * Dsh, 2, 9, Dsh], BF16)  # diag at rows Dsh:2Dsh
    convb = sb("convb_s", [Dsh, 2], F32)
    xproj_T = sb("xproj_T_s", [Dsh, K, XD], BF16)
    dtw02_s = sb("dtw02_s", [2 * R, K * Dsh], BF16)
    dtw1_s = sb("dtw1_s", [R, K * Dsh], BF16)
    dtb_s = sb("dtb96_s", [K * Dsh, 1], F32)
    A_col = sb("A_col_s", [128, K * G], F32)
    Ddiag_s = sb("Ddiag_s", [Dsh, Dsh], BF16)
    red_s = sb("red_s", [128, G, Dsh], BF16)
    seldd_s = sb("seldd_s", [DG, 128], BF16)
    selst = sb("selst_s", [128, 2, 4 * B], BF16)
    lnwb4 = sb("lnwb4_s", [128, 2], F32)
    opw4 = sb("opw4_s", [128, C], BF16)
    opw_s = sb("opw_s", [Dsh, 2, C], BF16)

    def allreduce(in_ap, out_ap):
        if cfg.fake_cc:
            nc.sync.dma_start(out=out_ap, in_=in_ap)
        else:
            nc.gpsimd.collective_compute(
                "AllReduce", OP.add, replica_groups=groups,
                ins=[in_ap.opt()], outs=[out_ap.opt()])

    with tile.TileContext(nc) as tc:
        # ---- load weights ----
        for dst, srcw in [
            (w_in_ms, t["w_in_ms"]), (w_in_pan, t["w_in_pan"]),
            (convd[Dsh:2 * Dsh, 0], t["conv_d_ms"].rearrange(
                "p (x d) -> p x d", d=Dsh)),
            (convd[Dsh:2 * Dsh, 1], t["conv_d_pan"].rearrange(
                "p (x d) -> p x d", d=Dsh)),
            (convb[:, 0:1], t["conv_b_ms"]), (convb[:, 1:2], t["conv_b_pan"]),
            (xproj_T, t["xproj_T"].rearrange("p (k x) -> p k x", x=XD)),
            (dtw02_s, t["dtw02"]), (dtw1_s, t["dtw1"]), (dtb_s, t["dtb96"]),
            (A_col, t["A_col"]), (Ddiag_s, t["Ddiag"]),
            (red_s, t["red"].rearrange("p (g d) -> p g d", d=Dsh)),
            (seldd_s, t["seldd"]),
            (selst, t["selst"].rearrange("p (x m) -> p x m", m=4 * B)),
            (lnwb4, t["lnwb4"]), (opw4, t["opw4"]),
            (opw_s, t["opw"].rearrange("p (s c) -> p s c", c=C)),
        ]:
            nc.sync.dma_start(out=dst, in_=srcw)

        # ================= Phase F: in_proj + conv + silu =================
        with tc.tile_pool(name="f_ps", bufs=2, space="PSUM") as f_ps, \
             tc.tile_pool(name="f_cv", bufs=2, space="PSUM") as f_cv, \
             tc.tile_pool(name="f_src", bufs=3) as f_src, \
             tc.tile_pool(name="f_st", bufs=3) as f_st, \
             tc.tile_pool(name="f_xpad", bufs=2) as f_xpad:
            for b in range(B):
                for s in range(2):
                    srcT = t["msf"] if s == 0 else t["panf"]
                    w_in = w_in_ms if s == 0 else w_in_pan
                    xpad = f_xpad.tile([2 * Dsh, Hp, Wp], BF16, tag="xpad")
                    nc.vector.memset(xpad[Dsh:2 * Dsh], 0.0)
                    for j in range(L // TOK):
                        mt = f_src.tile([C, TOK], BF16, tag="msrc")
                        nc.sync.dma_start(out=mt,
                                          in_=srcT[b, :, j * TOK:(j + 1) * TOK])
                        ps = f_ps.tile([2 * Dsh, TOK], F32, tag="fps")
                        mm(nc, ps, w_in, mt, start=True, stop=True)
                        rpc = TOK // W
                        nc.scalar.copy(
                            out=xpad[Dsh:2 * Dsh,
                                     1 + j * rpc:1 + (j + 1) * rpc, 1:1 + W],
                            in_=ps[Dsh:2 * Dsh, :].rearrange(
                                "p (r w) -> p r w", w=W))
                        zt = f_st.tile([Dsh, TOK], BF16, tag="zst")
                        nc.scalar.activation(out=zt, in_=ps[0:Dsh, :],
                                             func=AF.Silu)
                        nc.sync.dma_start(
                            out=zs_dram[:, 2 * b + s, j * TOK:(j + 1) * TOK],
                            in_=zt)
                    # conv: 9 accumulated diag matmuls per row-chunk
                    for j in range(H // CRW):
                        cps = f_cv.tile([Dsh, CRW * W], F32, tag="cps")
                        for tap in range(9):
                            ky, kx = tap // 3, tap % 3
                            rhs = xpad[Dsh:2 * Dsh,
                                       ky + j * CRW: ky + (j + 1) * CRW,
                                       kx:kx + W]
                            nc.tensor.matmul(cps, convd[Dsh:2 * Dsh, s, tap, :],
                                             rhs,
                                             start=(tap == 0), stop=(tap == 8),
                                             skip_group_check=True)
                        ct = f_st.tile([Dsh, CRW * W], BF16, tag="cst")
                        nc.scalar.activation(out=ct, in_=cps, func=AF.Silu,
                                             bias=convb[:, s:s + 1])
                        # interleaved write: position (j*CRW*W + t)*2 + s
                        nc.sync.dma_start(
                            out=dram_ap(inter_dram,
                                        b * L2 + j * CRW * W * 2 + s,
                                        [[B * L2, Dsh], [2, CRW * W]]),
                            in_=ct)

        # ============ Phase X + AR + scan, shared PSUM budget ============
        with tc.tile_pool(name="x_ps", bufs=2, space="PSUM") as x_ps, \
             tc.tile_pool(name="s_dt_ps", bufs=2, space="PSUM") as s_dt_ps, \
             tc.tile_pool(name="s_nd_ps", bufs=2, space="PSUM") as s_nd_ps, \
             tc.tile_pool(name="s_y_ps", bufs=1, space="PSUM") as s_y_ps, \
             tc.tile_pool(name="x_ib", bufs=3) as x_ib, \
             tc.tile_pool(name="x_st", bufs=3) as x_st, \
             tc.tile_pool(name="s_io", bufs=2) as s_io, \
             tc.tile_pool(name="s_g8", bufs=1) as s_g8, \
             tc.tile_pool(name="s_dt", bufs=2) as s_dt, \
             tc.tile_pool(name="s_a", bufs=3) as s_a, \
             tc.tile_pool(name="s_b", bufs=3) as s_b, \
             tc.tile_pool(name="s_h", bufs=2) as s_h, \
             tc.tile_pool(name="s_hc", bufs=3) as s_hc, \
             tc.tile_pool(name="s_rep", bufs=3) as s_rep:

            def phase_x(b):
                for j in range(L2 // PC):
                    ibt = x_ib.tile([Dsh, PC], BF16, tag="xib")
                    nc.sync.dma_start(
                        out=ibt, in_=inter_dram[:, b, j * PC:(j + 1) * PC])
                    for k in range(K):
                        st = x_st.tile([XD, PC], BF16, tag="xst")
                        for h in range(PC // HMM):
                            ps = x_ps.tile([XD, HMM], F32, tag="xps")
                            nc.tensor.matmul(
                                ps, xproj_T[:, k, :],
                                ibt[:, h * HMM:(h + 1) * HMM],
                                start=True, stop=True, skip_group_check=True)
                            nc.scalar.copy(out=st[:, h * HMM:(h + 1) * HMM],
                                           in_=ps)
                        nc.sync.dma_start(
                            out=xdbl_part[b, k * XD:(k + 1) * XD,
                                          j * PC:(j + 1) * PC],
                            in_=st)

            # X(b0); AR(b0); X(b1); scan(b0); AR(b1); scan(b1) — the AR(b1)
            # trigger sits after scan(b0)'s Pool muls so the in-order Pool
            # queue doesn't stall on X(b1) completion before scanning b0.
            phase_x(0)
            allreduce(xdbl_part[0], xdbl_full[0])
            phase_x(1)

            h_prev = {}

            def phase_scan(b):
                for ch in range(NCH):
                    par = ch % 2
                    cs = slice(ch * Tc, (ch + 1) * Tc)
                    mcs = slice(L2 - (ch + 1) * Tc, L2 - ch * Tc)
                    xfb = xdbl_full[b]
                    xoff = b * (K * XD * L2)

                    ibc = s_io.tile([Dsh, Tc], BF16, tag="ibc")
                    nc.sync.dma_start(out=ibc, in_=inter_dram[:, b, cs])
                    ibmc = s_io.tile([Dsh, Tc], BF16, tag="ibmc")
                    nc.sync.dma_start(out=ibmc, in_=inter_dram[:, b, mcs])
                    # dt rows k0+k2 fwd, k1 mirror
                    stg16 = s_io.tile([2 * R, Tc], BF16, tag="stg16")
                    nc.sync.dma_start(
                        out=stg16,
                        in_=dram_ap(xdbl_full, xoff + ch * Tc,
                                    [[2 * XD * L2, 2], [L2, R], [1, Tc]]))
                    stg8 = s_io.tile([R, Tc], BF16, tag="stg8")
                    nc.sync.dma_start(
                        out=stg8, in_=xfb[XD:XD + R, mcs])
                    stg8R = s_io.tile([R, Tc], BF16, tag="stg8R")
                    nc.vector.tensor_copy(out=stg8R, in_=stg8[:, ::-1])
                    # k1 B/C rows (mirror) -> reversed -> DRAM bounce
                    bcF = s_io.tile([2 * N, Tc], BF16, tag="bcF")
                    nc.sync.dma_start(
                        out=bcF, in_=xfb[XD + R:XD + R + 2 * N, mcs])
                    bcR = s_io.tile([2 * N, Tc], BF16, tag="bcR")
                    nc.vector.tensor_copy(out=bcR, in_=bcF[:, ::-1])
                    nc.sync.dma_start(out=brc_dram[:, par, :], in_=bcR)
                    # fused dt: Exp then Ln(1+x) = softplus
                    e96 = s_dt.tile([K * Dsh, Tc], F32, tag="e96")
                    for h in range(Tc // HMM):
                        hs = slice(h * HMM, (h + 1) * HMM)
                        dtp = s_dt_ps.tile([K * Dsh, HMM], F32, tag="dtp")
                        nc.tensor.matmul(dtp, dtw02_s, stg16[:, hs],
                                         start=True, stop=False,
                                         skip_group_check=True)
                        nc.tensor.matmul(dtp, dtw1_s, stg8R[:, hs],
                                         start=False, stop=True,
                                         skip_group_check=True)
                        nc.scalar.activation(out=e96[:, hs], in_=dtp,
                                             func=AF.Exp,
                                             bias=dtb_s[:, 0:1], scale=1.0)
                    dt96 = s_dt.tile([K * Dsh, Tc], BF16, tag="dt96")
                    nc.scalar.activation(out=dt96, in_=e96, func=AF.Ln,
                                         bias=1.0)
                    # x3 = [ibc; rev(ibmc); ibc] then one fused mul
                    # (tensor_tensor requires same start partition on all
                    # operands; copies don't)
                    x3 = s_dt.tile([K * Dsh, Tc], BF16, tag="x3")
                    nc.vector.tensor_copy(out=x3[0:Dsh], in_=ibc)
                    nc.vector.tensor_copy(out=x3[Dsh:2 * Dsh],
                                          in_=ibmc[:, ::-1])
                    nc.vector.tensor_copy(out=x3[2 * Dsh:3 * Dsh], in_=ibc)
                    dtx96 = s_dt.tile([K * Dsh, Tc], BF16, tag="dtx96")
                    nc.vector.tensor_mul(dtx96, dt96, x3)
                    # bounce dt96 -> [8, 12*Tc] base-0 staging
                    nc.sync.dma_start(out=dtg_dram[par], in_=dt96)
                    g8 = s_g8.tile([DG, K * G, Tc], BF16, tag="g8")
                    nc.sync.dma_start(
                        out=g8,
                        in_=dram_ap(dtg_dram, par * (K * Dsh * Tc),
                                    [[Tc, DG], [DG * Tc, K * G], [1, Tc]]))
                    # B/C replicated loads
                    brep = {}
                    crep = {}
                    for k in (0, 2):
                        bk = s_rep.tile([128, Tc], BF16, tag=f"brep{k}")
                        nc.scalar.dma_start(
                            out=bk, in_=rep_dram(
                                xdbl_full,
                                xoff + (k * XD + R) * L2 + ch * Tc,
                                L2, N, DG, 1, Tc))
                        brep[k] = bk
                        ck = s_rep.tile([128, Tc], BF16, tag=f"crep{k}")
                        nc.scalar.dma_start(
                            out=ck, in_=rep_dram(
                                xdbl_full,
                                xoff + (k * XD + R + N) * L2 + ch * Tc,
                                L2, N, DG, 1, Tc))
                        crep[k] = ck
                    b1 = s_rep.tile([128, Tc], BF16, tag="brep1")
                    nc.scalar.dma_start(
                        out=b1, in_=rep_dram(brc_dram, par * Tc,
                                             2 * Tc, N, DG, 1, Tc))
                    brep[1] = b1
                    c1 = s_rep.tile([128, Tc], BF16, tag="crep1")
                    nc.scalar.dma_start(
                        out=c1, in_=rep_dram(brc_dram, (2 * N + par) * Tc,
                                             2 * Tc, N, DG, 1, Tc))
                    crep[1] = c1

                    y_ps = s_y_ps.tile([2 * Dsh, Tc], F32, tag="yps")
                    # D*x folded into the y02 rows (start of accumulation)
                    for h in range(Tc // HMM):
                        hs = slice(h * HMM, (h + 1) * HMM)
                        nc.tensor.matmul(y_ps[0:Dsh, hs], Ddiag_s,
                                         ibc[:, hs], start=True, stop=False,
                                         skip_group_check=True)

                    for k in range(K):
                        for g in range(G):
                            ci = k * G + g
                            a_t = s_a.tile([128, Tc], BF16, tag="a")
                            for h in range(Tc // HMM):
                                hs = slice(h * HMM, (h + 1) * HMM)
                                ndp = s_nd_ps.tile([128, HMM], F32, tag="ndp")
                                nc.tensor.matmul(ndp, seldd_s,
                                                 g8[:, ci, hs],
                                                 start=True, stop=True,
                                                 skip_group_check=True)
                                nc.scalar.activation(
                                    out=a_t[:, hs], in_=ndp, func=AF.Exp,
                                    scale=A_col[:, ci:ci + 1])
                            dtxr = s_rep.tile([128, Tc], BF16, tag="dxr")
                            nc.sync.dma_start(
                                out=dtxr,
                                in_=rep_sbuf(
                                    dtx96[k * Dsh + g * DG:
                                          k * Dsh + (g + 1) * DG, :], N))
                            b_t = s_b.tile([128, Tc], BF16, tag="bt")
                            nc.gpsimd.tensor_mul(b_t, dtxr, brep[k])
                            h_t = s_h.tile([128, Tc], BF16, tag=f"h{ci}")
                            key = (b, ci)
                            init = 0.0 if ch == 0 else \
                                h_prev[key][:, Tc - 1:Tc]
                            nc.vector.tensor_tensor_scan(
                                h_t, a_t, b_t, init, OP.mult, OP.add)
                            h_prev[key] = h_t
                            hc = s_hc.tile([128, Tc], BF16, tag="hc")
                            if ci in (0, 6):
                                nc.gpsimd.tensor_mul(hc, h_t, crep[k])
                            else:
                                nc.vector.tensor_mul(hc, h_t, crep[k])
                            ro = 0 if k != 1 else Dsh
                            last = (k == 2 and g == G - 1) if k != 1 \
                                else (g == G - 1)
                            for h in range(Tc // HMM):
                                hs = slice(h * HMM, (h + 1) * HMM)
                                nc.tensor.matmul(
                                    y_ps[ro:ro + Dsh, hs], red_s[:, g, :],
                                    hc[:, hs],
                                    start=(k == 1 and g == 0),
                                    stop=last, skip_group_check=True)
                    yev = s_io.tile([2 * Dsh, Tc], BF16, tag="yev")
                    nc.scalar.copy(out=yev, in_=y_ps)
                    nc.sync.dma_start(
                        out=dram_ap(yev_dram, b * 2 * L2 + ch * Tc,
                                    [[L2, 2], [B * 2 * L2, Dsh], [1, Tc]]),
                        in_=yev)

            phase_scan(0)
            allreduce(xdbl_part[1], xdbl_full[1])
            phase_scan(1)

        # ============ Phase L: LN stats + allreduce + apply + out_proj ======
        half = 2 * B * L
        with tc.tile_pool(name="l_ps", bufs=2, space="PSUM") as l_ps, \
             tc.tile_pool(name="l_one", bufs=1) as l_one, \
             tc.tile_pool(name="l_sq", bufs=3) as l_sq, \
             tc.tile_pool(name="l_stg", bufs=4) as l_stg:
            ydp = l_one.tile([128, L], BF16, tag="ydp")
            MC = 1024
            for b in range(B):
                for jj in range(L2 // MC):
                    yft = l_stg.tile([Dsh, MC], BF16, tag="yft")
                    nc.sync.dma_start(
                        out=yft, in_=yev_dram[:, b, 0, jj * MC:(jj + 1) * MC])
                    y1t = l_stg.tile([Dsh, MC], BF16, tag="y1t")
                    nc.sync.dma_start(
                        out=y1t,
                        in_=yev_dram[:, b, 1, L2 - (jj + 1) * MC:L2 - jj * MC])
                    ym = l_stg.tile([Dsh, MC], BF16, tag="ym")
                    nc.vector.tensor_add(ym, yft, y1t[:, ::-1])
                    tok = slice(jj * MC // 2, (jj + 1) * MC // 2)
                    nc.vector.tensor_copy(
                        out=ydp[(2 * b) * Dsh:(2 * b + 1) * Dsh, tok],
                        in_=ym[:, 0::2])
                    nc.vector.tensor_copy(
                        out=ydp[(2 * b + 1) * Dsh:(2 * b + 2) * Dsh, tok],
                        in_=ym[:, 1::2])
            for j in range(nLch):
                js = slice(j * TcL, (j + 1) * TcL)
                sqp = l_sq.tile([128, TcL], BF16, tag="sqp")
                nc.vector.tensor_mul(sqp, ydp[:, js], ydp[:, js])
                sp = l_ps.tile([4 * B, TcL], F32, tag="sps")
                mm(nc, sp, selst[:, 0, :], ydp[:, js], start=True, stop=False)
                mm(nc, sp, selst[:, 1, :], sqp, start=False, stop=True)
                stg = l_stg.tile([4 * B, TcL], F32, tag="sstg2")
                nc.scalar.copy(out=stg, in_=sp)
                nc.sync.dma_start(
                    out=stats_part.rearrange("a x l -> (a x) l")[:, js], in_=stg)
            allreduce(stats_part, stats_full)
            s1f = l_one.tile([64, SC], F32, tag="s1f")
            s2f = l_one.tile([64, SC], F32, tag="s2f")
            flat = stats_full.rearrange("a x l -> (a x l)")
            nc.sync.dma_start(
                out=s1f, in_=flat[0:half].rearrange("(p c) -> p c", p=64))
            nc.sync.dma_start(
                out=s2f, in_=flat[half:2 * half].rearrange("(p c) -> p c", p=64))
            mu_t = l_one.tile([64, SC], F32, tag="mu_t")
            var_t = l_one.tile([64, SC], F32, tag="var_t")
            musq = l_one.tile([64, SC], F32, tag="musq")
            eps_t = l_one.tile([64, 1], F32, tag="eps_t")
            nc.vector.memset(eps_t, 1e-5)
            nc.vector.tensor_scalar_mul(mu_t, s1f, 1.0 / D)
            nc.vector.tensor_scalar_mul(var_t, s2f, 1.0 / D)
            nc.vector.tensor_mul(musq, mu_t, mu_t)
            nc.vector.tensor_sub(var_t, var_t, musq)
            nc.scalar.activation(out=var_t, in_=var_t, func=AF.Sqrt, bias=eps_t)
            nc.vector.reciprocal(out=s1f, in_=var_t)          # alpha
            nc.vector.tensor_mul(s2f, mu_t, s1f)
            nc.vector.tensor_scalar_mul(s2f, s2f, -1.0)       # beta
            nc.sync.dma_start(
                out=ab_dram.rearrange("a x l -> (a x l)")[0:half].rearrange(
                    "(p c) -> p c", p=64), in_=s1f)
            nc.sync.dma_start(
                out=ab_dram.rearrange("a x l -> (a x l)")[half:2 * half].rearrange(
                    "(p c) -> p c", p=64), in_=s2f)

            # apply LN + gate + out_proj; stream s=0 fully, RS, then s=1, RS
            with tc.tile_pool(name="l_rep", bufs=3) as l_rep, \
                 tc.tile_pool(name="l_t", bufs=3) as l_t, \
                 tc.tile_pool(name="l_fg", bufs=1) as l_fg, \
                 tc.tile_pool(name="o_st", bufs=2) as o_st, \
                 tc.tile_pool(name="o_ps", bufs=2, space="PSUM") as o_ps:
                fgs = []
                for j in range(nLch):
                    js = slice(j * TcL, (j + 1) * TcL)
                    zcp = l_rep.tile([128, TcL], BF16, tag="zcp")
                    nc.sync.dma_start(
                        out=zcp,
                        in_=bass.AP(tensor=zs_dram.tensor,
                                    offset=zs_dram.offset + j * TcL,
                                    ap=[[L, 2 * B], [2 * B * L, Dsh],
                                        [1, TcL]]))
                    zwp = l_t.tile([128, TcL], BF16, tag="zwp")
                    bzp = l_t.tile([128, TcL], BF16, tag="bzp")
                    nc.vector.tensor_scalar_mul(zwp, zcp, lnwb4[:, 0:1])
                    nc.vector.tensor_scalar_mul(bzp, zcp, lnwb4[:, 1:2])
                    arp = l_rep.tile([128, TcL], F32, tag="arp")
                    brp = l_rep.tile([128, TcL], F32, tag="brp")
                    nc.sync.dma_start(
                        out=arp,
                        in_=bass.AP(tensor=ab_dram.tensor,
                                    offset=ab_dram.offset + j * TcL,
                                    ap=[[L, 2 * B], [0, Dsh], [1, TcL]]))
                    nc.scalar.dma_start(
                        out=brp,
                        in_=bass.AP(tensor=ab_dram.tensor,
                                    offset=ab_dram.offset + half + j * TcL,
                                    ap=[[L, 2 * B], [0, Dsh], [1, TcL]]))
                    t1 = l_t.tile([128, TcL], BF16, tag="t1")
                    nc.vector.tensor_mul(t1, ydp[:, js], arp)
                    t2 = l_t.tile([128, TcL], BF16, tag="t2")
                    nc.vector.tensor_add(t2, t1, brp)
                    t3 = l_t.tile([128, TcL], BF16, tag="t3")
                    nc.vector.tensor_mul(t3, t2, zwp)
                    fgp = l_fg.tile([128, TcL], BF16, tag=f"fgp{j}")
                    nc.vector.tensor_add(fgp, t3, bzp)
                    fgs.append(fgp)
                for s in range(2):
                    for j in range(nLch):
                        js = slice(j * TcL, (j + 1) * TcL)
                        fgp = fgs[j]
                        for b in range(B):
                            bs = b * 2 + s
                            qs = slice(bs * Dsh, (bs + 1) * Dsh)
                            if bs * Dsh in (0, 32, 64):
                                lhs = opw4[qs, :]
                                rhs = fgp[qs, :]
                            else:
                                stq = o_st.tile([Dsh, TcL], BF16, tag="stq")
                                nc.sync.dma_start(out=stq, in_=fgp[qs, :])
                                lhs = opw_s[:, s, :]
                                rhs = stq
                            ops = o_ps.tile([C, TcL], F32, tag="ops")
                            mm(nc, ops, lhs, rhs, start=True, stop=True)
                            ost = o_st.tile([C, TcL], F32, tag="ost")
                            nc.scalar.copy(out=ost, in_=ops)
                            nc.sync.dma_start(out=outp_part[s, b, :, js],
                                              in_=ost)
                    # ReduceScatter this stream's half
                    hflat = flat_out // 2
                    hchunk = chunk_o // 2
                    pflat = outp_part.rearrange("s b c l -> (s b c l)")
                    if cfg.fake_cc:
                        nc.sync.dma_start(
                            out=out_chunk[s * hchunk:(s + 1) * hchunk],
                            in_=pflat[s * hflat:s * hflat + hchunk])
                    else:
                        nc.gpsimd.collective_compute(
                            "ReduceScatter", OP.add, replica_groups=groups,
                            ins=[pflat[s * hflat:(s + 1) * hflat].opt()],
                            outs=[outp_rs[s * hchunk:(s + 1) * hchunk].opt()])
                        nc.sync.dma_start(
                            out=out_chunk[s * hchunk:(s + 1) * hchunk],
                            in_=outp_rs[s * hchunk:(s + 1) * hchunk])

    nc.compile()
    return nc


# ---------------------------------------------------------------------------
# public entry point
# ---------------------------------------------------------------------------

_CACHE = {}


def _get_nc(cfg: Cfg):
    if cfg not in _CACHE:
        _CACHE[cfg] = build_nc(cfg)
    return _CACHE[cfg]


def kernel(**inputs):
    cfg = CFG
    nc = _get_nc(cfg)
    in_maps = host_prep(cfg, inputs)
    res = bass_utils.run_bass_kernel_spmd(
        nc, in_maps, core_ids=list(range(cfg.n_cores)))
    return assemble_outputs(cfg, res.results)


def assemble_outputs(cfg, results):
    """Each core returns its ReduceScatter chunks: [ms-chunk | pan-chunk]."""
    B, C, L = cfg.B, cfg.C, cfg.L
    hflat = B * C * L
    hchunk = hflat // cfg.n_cores
    ms_flat = np.zeros(hflat, np.float32)
    pan_flat = np.zeros(hflat, np.float32)
    for r in range(cfg.n_cores):
        ck = np.asarray(results[r]["out_chunk"], np.float32)
        ms_flat[r * hchunk:(r + 1) * hchunk] = ck[0:hchunk]
        pan_flat[r * hchunk:(r + 1) * hchunk] = ck[hchunk:2 * hchunk]
    out_ms = ms_flat.reshape(B, C, cfg.H, cfg.W)
    out_pan = pan_flat.reshape(B, C, cfg.H, cfg.W)
    return (out_ms, out_pan)


# revision 12
# speedup vs baseline: 2.2131x; 2.2131x over previous
"""Trainium2 Bass kernel for nn_DSSM_57629871178390 (dual-stream Mamba/DSSM block).

Sharding: d_inner=256 split 8 ways across cores (Dsh=32 channels each). The
selective scan runs on the DVE via tensor_tensor_scan with 128-partition
tiles laid out as (dd=8 d-values x n=16 states); K=3 x G=4 tiles per (b,chunk).

v2 restructure vs baseline:
- softplus(dt) via Exp+Ln (one ACT table with the scan's Exp -> no table loads)
- dt for all 3 scan routes fused into one [96,Tc] matmul+ACT chain
- ndt replication via PE matmul from a [8,12*Tc] DRAM-bounced staging tile
- b_t muls on the Pool engine (GpSimd), hc muls mostly on DVE
- inter stored interleaved in DRAM (no per-load de-interleave copies)
- per-batch AllReduce of x_dbl, overlapped with the other batch's scan
- ReduceScatter split per stream (ms/pan) to overlap with out_proj
- scan carry chained via previous h tile's last column (no copy ops)

Self-contained: hardcodes all shapes; imports numpy + concourse (available
at /opt/trn_rl_repo inside the container).
"""

import sys
from dataclasses import dataclass

import numpy as np

if "/opt/trn_rl_repo" not in sys.path:
    sys.path.insert(0, "/opt/trn_rl_repo")

import ml_dtypes  # noqa: E402

import concourse.bass as bass  # noqa: E402
import concourse.bacc as bacc  # noqa: E402
import concourse.tile as tile  # noqa: E402
import concourse.mybir as mybir  # noqa: E402
from concourse import bass_utils  # noqa: E402

F32 = mybir.dt.float32
BF16 = mybir.dt.bfloat16
AF = mybir.ActivationFunctionType
OP = mybir.AluOpType
NPBF16 = ml_dtypes.bfloat16


@dataclass(frozen=True)
class Cfg:
    B: int = 2
    C: int = 128
    H: int = 64
    W: int = 64
    D: int = 256          # d_inner
    N: int = 16           # d_state
    R: int = 8            # dt_rank
    K: int = 3
    n_cores: int = 8
    fake_cc: bool = False   # replace collectives with plain DMA (profiling)

    @property
    def L(self):
        return self.H * self.W

    @property
    def L2(self):
        return 2 * self.L

    @property
    def Dsh(self):
        return self.D // self.n_cores

    @property
    def DG(self):
        return 128 // self.N       # d-values per 128-partition tile (8)

    @property
    def G(self):
        return self.Dsh // self.DG  # d-groups per core (4)

    @property
    def Tc(self):
        return 1024                 # scan chunk

    @property
    def TOK(self):
        return min(512, self.L)     # in_proj token chunk

    @property
    def CR(self):
        return max(1, min(512 // self.W, self.H))  # conv rows per chunk

    @property
    def TcL(self):
        return min(512, self.L)     # LN/out_proj chunk


CFG = Cfg()
MMF = 512  # max moving free dim per matmul


# ---------------------------------------------------------------------------
# Host-side preparation of per-core input maps
# ---------------------------------------------------------------------------

def host_prep(cfg: Cfg, inputs: dict) -> list:
    B, C, H, W = cfg.B, cfg.C, cfg.H, cfg.W
    D, N, R, K = cfg.D, cfg.N, cfg.R, cfg.K
    Dsh, G, DG, L = cfg.Dsh, cfg.G, cfg.DG, cfg.L

    ms = np.asarray(inputs["ms"], np.float32).reshape(B, C, L)
    pan = np.asarray(inputs["pan"], np.float32).reshape(B, C, L)
    w_ms = np.asarray(inputs["in_proj_ms_w"], np.float32)
    w_pan = np.asarray(inputs["in_proj_pan_w"], np.float32)
    cw_ms = np.asarray(inputs["conv_ms_w"], np.float32)
    cb_ms = np.asarray(inputs["conv_ms_b"], np.float32)
    cw_pan = np.asarray(inputs["conv_pan_w"], np.float32)
    cb_pan = np.asarray(inputs["conv_pan_b"], np.float32)
    xpw = np.asarray(inputs["x_proj_weight"], np.float32)
    dtw = np.asarray(inputs["dt_projs_weight"], np.float32)
    dtb = np.asarray(inputs["dt_projs_bias"], np.float32)
    A_logs = np.asarray(inputs["A_logs"], np.float32)
    Ds = np.asarray(inputs["Ds"], np.float32)
    lnw_v = np.asarray(inputs["out_norm_vis_w"], np.float32)
    lnb_v = np.asarray(inputs["out_norm_vis_b"], np.float32)
    lnw_i = np.asarray(inputs["out_norm_inf_w"], np.float32)
    lnb_i = np.asarray(inputs["out_norm_inf_b"], np.float32)
    opw_ms = np.asarray(inputs["out_proj_ms_w"], np.float32)
    opw_pan = np.asarray(inputs["out_proj_pan_w"], np.float32)

    A = np.exp(A_logs).reshape(K, D, N)
    Dsum = Ds.reshape(K, D).sum(0)

    # partition layout of scan tiles: p = dd*N + n
    red = np.zeros((128, G * Dsh), NPBF16)
    for g in range(G):
        for p in range(128):
            red[p, g * Dsh + DG * g + (p // N)] = 1.0

    selst_h = np.zeros((128, 2, 4 * B), NPBF16)
    for pi in range(128):
        selst_h[pi, 0, pi // Dsh] = 1.0          # S1 rows
        selst_h[pi, 1, 2 * B + pi // Dsh] = 1.0  # S2 rows
    seldd = np.zeros((DG, 128), NPBF16)
    for pi in range(128):
        seldd[pi // N, pi] = 1.0

    shared = {
        "seldd": seldd,
        "selst": selst_h.reshape(128, 2 * 4 * B),
        "msf": ms.astype(NPBF16),
        "panf": pan.astype(NPBF16),
        "red": red,
    }

    maps = []
    for c in range(cfg.n_cores):
        dsl = slice(Dsh * c, Dsh * (c + 1))
        m = dict(shared)
        # cols 0:Dsh = z rows (W[D+dsl]), cols Dsh:2Dsh = x rows (W[dsl])
        m["w_in_ms"] = np.ascontiguousarray(
            np.concatenate([w_ms[D + Dsh * c: D + Dsh * (c + 1)].T,
                            w_ms[dsl].T], 1)).astype(NPBF16)
        m["w_in_pan"] = np.ascontiguousarray(
            np.concatenate([w_pan[D + Dsh * c: D + Dsh * (c + 1)].T,
                            w_pan[dsl].T], 1)).astype(NPBF16)
        cd_ms = np.zeros((Dsh, 9 * Dsh), NPBF16)
        cd_pan = np.zeros((Dsh, 9 * Dsh), NPBF16)
        for t in range(9):
            ky, kx = t // 3, t % 3
            for i in range(Dsh):
                cd_ms[i, t * Dsh + i] = cw_ms[Dsh * c + i, 0, ky, kx]
                cd_pan[i, t * Dsh + i] = cw_pan[Dsh * c + i, 0, ky, kx]
        m["conv_d_ms"] = cd_ms
        m["conv_d_pan"] = cd_pan
        m["conv_b_ms"] = cb_ms[dsl].reshape(Dsh, 1).astype(np.float32)
        m["conv_b_pan"] = cb_pan[dsl].reshape(Dsh, 1).astype(np.float32)
        m["xproj_T"] = np.ascontiguousarray(
            xpw[:, :, dsl].transpose(2, 0, 1).reshape(Dsh, K * (R + 2 * N))
        ).astype(NPBF16)
        # fused dt projection: out partition = k*Dsh + dl
        dtw02 = np.zeros((2 * R, 3 * Dsh), NPBF16)
        dtw1 = np.zeros((R, 3 * Dsh), NPBF16)
        for r in range(R):
            for dl in range(Dsh):
                dtw02[r, dl] = dtw[0, Dsh * c + dl, r]
                dtw02[R + r, 2 * Dsh + dl] = dtw[2, Dsh * c + dl, r]
                dtw1[r, Dsh + dl] = dtw[1, Dsh * c + dl, r]
        m["dtw02"] = dtw02
        m["dtw1"] = dtw1
        m["dtb96"] = np.ascontiguousarray(
            dtb[:, dsl].reshape(K * Dsh, 1)).astype(np.float32)
        acol = np.zeros((128, K * G), np.float32)
        for k in range(K):
            for g in range(G):
                for p in range(128):
                    dd, n = p // N, p % N
                    acol[p, k * G + g] = -A[k, Dsh * c + DG * g + dd, n]
        m["A_col"] = acol
        Ddiag = np.zeros((Dsh, Dsh), NPBF16)
        for i in range(Dsh):
            Ddiag[i, i] = Dsum[Dsh * c + i]
        m["Ddiag"] = Ddiag
        lw4 = np.zeros((128, 2), np.float32)
        for bs in range(2 * B):
            sle = bs % 2
            lw4[bs * Dsh:(bs + 1) * Dsh, 0] = (lnw_v if sle == 0 else lnw_i)[dsl]
            lw4[bs * Dsh:(bs + 1) * Dsh, 1] = (lnb_v if sle == 0 else lnb_i)[dsl]
        m["lnwb4"] = lw4
        o4 = np.zeros((128, C), NPBF16)
        for bs in range(2 * B):
            sle = bs % 2
            o4[bs * Dsh:(bs + 1) * Dsh] = (
                (opw_ms if sle == 0 else opw_pan)[:, dsl].T).astype(NPBF16)
        m["opw4"] = o4
        m["opw"] = np.ascontiguousarray(
            np.concatenate([opw_ms[:, dsl].T, opw_pan[:, dsl].T], 1)
        ).astype(NPBF16)  # (Dsh, 2C)
        maps.append(m)
    return maps


# ---------------------------------------------------------------------------
# AP helpers
# ---------------------------------------------------------------------------

def rep_sbuf(ap2d, n_inner):
    """SBUF [P, F] slice -> src AP for dest [P*n_inner, F] with each source
    partition repeated n_inner times (dest p = i*n_inner + j reads src i)."""
    pairs = [list(x) for x in ap2d.ap]
    assert len(pairs) == 2
    return bass.AP(tensor=ap2d.tensor, offset=ap2d.offset,
                   ap=[pairs[0], [0, n_inner], pairs[1]])


def rep_dram(tensor_ap, offset, row_stride, n_rows, n_rep, fstep, fcount):
    """DRAM source AP replicating a [n_rows, fcount] block so that dest
    partition p = j*n_rows + r reads row r (rep index j outer)."""
    return bass.AP(tensor=tensor_ap.tensor, offset=tensor_ap.offset + offset,
                   ap=[[0, n_rep], [row_stride, n_rows], [fstep, fcount]])


def dram_ap(tensor_ap, offset, pairs):
    return bass.AP(tensor=tensor_ap.tensor, offset=tensor_ap.offset + offset,
                   ap=pairs)


def mm(nc, out_ps, lhsT, rhs, start, stop, maxf=MMF):
    """Matmul with moving-free-dim splitting. out/rhs 2D [P, F]."""
    F = rhs.shape[-1]
    if F <= maxf:
        nc.tensor.matmul(out_ps, lhsT, rhs, start=start, stop=stop,
                         skip_group_check=True)
        return
    assert F % maxf == 0
    for i in range(F // maxf):
        nc.tensor.matmul(out_ps[:, i * maxf:(i + 1) * maxf], lhsT,
                         rhs[:, i * maxf:(i + 1) * maxf], start=start, stop=stop,
                         skip_group_check=True)


# ---------------------------------------------------------------------------
# Bass program builder
# ---------------------------------------------------------------------------

def build_nc(cfg: Cfg):
    B, C, H, W = cfg.B, cfg.C, cfg.H, cfg.W
    D, N, R, K = cfg.D, cfg.N, cfg.R, cfg.K
    Dsh, G, DG = cfg.Dsh, cfg.G, cfg.DG
    L, L2, Tc = cfg.L, cfg.L2, cfg.Tc
    TOK, CRW, TcL = cfg.TOK, cfg.CR, cfg.TcL
    NCH = L2 // Tc               # scan chunks per batch (16)
    PC = 1024                    # x_dbl phase chunk
    XD = R + 2 * N               # 40
    Hp, Wp = H + 2, W + 2
    groups = [list(range(cfg.n_cores))]
    nLch = L // TcL
    SC = 2 * B * L // 64         # stats cols for [64, SC] view
    HMM = 512

    nc = bacc.Bacc("TRN2", target_bir_lowering=False, debug=False,
                   enable_asserts=False, num_devices=cfg.n_cores)

    t = {}

    def inp(name, shape, dt):
        t[name] = nc.dram_tensor(name, shape, dt, kind="ExternalInput").ap()

    inp("msf", [B, C, L], BF16)
    inp("panf", [B, C, L], BF16)
    inp("w_in_ms", [C, 2 * Dsh], BF16)    # cols 0:Dsh z-rows, Dsh:2Dsh x-rows
    inp("w_in_pan", [C, 2 * Dsh], BF16)
    inp("conv_d_ms", [Dsh, 9 * Dsh], BF16)
    inp("conv_d_pan", [Dsh, 9 * Dsh], BF16)
    inp("conv_b_ms", [Dsh, 1], F32)
    inp("conv_b_pan", [Dsh, 1], F32)
    inp("xproj_T", [Dsh, K * XD], BF16)
    inp("dtw02", [2 * R, K * Dsh], BF16)
    inp("dtw1", [R, K * Dsh], BF16)
    inp("dtb96", [K * Dsh, 1], F32)
    inp("A_col", [128, K * G], F32)       # NEGATIVE -exp(A_logs)
    inp("Ddiag", [Dsh, Dsh], BF16)
    inp("red", [128, G * Dsh], BF16)
    inp("seldd", [DG, 128], BF16)
    inp("selst", [128, 2 * 4 * B], BF16)
    inp("lnwb4", [128, 2], F32)
    inp("opw4", [128, C], BF16)
    inp("opw", [Dsh, 2 * C], BF16)

    flat_out = 2 * B * C * L
    chunk_o = flat_out // cfg.n_cores
    out_chunk = nc.dram_tensor("out_chunk", [chunk_o], F32,
                               kind="ExternalOutput").ap()

    shsp = "Shared" if cfg.n_cores > 4 else "Local"
    inter_dram = nc.dram_tensor("inter_dram", [Dsh, B, L2], BF16,
                                kind="Internal").ap()
    zs_dram = nc.dram_tensor("zs_dram", [Dsh, 2 * B, L], BF16,
                             kind="Internal").ap()
    xdbl_part = nc.dram_tensor("xdbl_part", [B, K * XD, L2], BF16,
                               kind="Internal").ap()
    xdbl_full = nc.dram_tensor("xdbl_full", [B, K * XD, L2], BF16,
                               kind="Internal", addr_space=shsp).ap()
    dtg_dram = nc.dram_tensor("dtg_dram", [2, K * Dsh, Tc], BF16,
                              kind="Internal").ap()
    brc_dram = nc.dram_tensor("brc_dram", [2 * N, 2, Tc], BF16,
                              kind="Internal").ap()
    yev_dram = nc.dram_tensor("yev_dram", [Dsh, B, 2, L2], BF16,
                              kind="Internal").ap()
    stats_part = nc.dram_tensor("stats_part", [2, 2 * B, L], F32,
                                kind="Internal").ap()
    stats_full = nc.dram_tensor("stats_full", [2, 2 * B, L], F32,
                                kind="Internal", addr_space=shsp).ap()
    ab_dram = nc.dram_tensor("ab_dram", [2, 2 * B, L], F32,
                             kind="Internal").ap()
    outp_part = nc.dram_tensor("outp_part", [2, B, C, L], F32,
                               kind="Internal").ap()
    outp_rs = nc.dram_tensor("outp_rs", [2 * B * C * L // cfg.n_cores], F32,
                             kind="Internal").ap()

    # persistent SBUF weights
    def sb(name, shape, dt):
        return nc.alloc_sbuf_tensor(name, shape, dt).ap()

    w_in_ms = sb("w_in_ms_s", [C, 2 * Dsh], BF16)
    w_in_pan = sb("w_in_pan_s", [C, 2 * Dsh], BF16)
    convd = sb("convd_s", [2 * Dsh, 2, 9, Dsh], BF16)  # diag at rows Dsh:2Dsh
    convb = sb("convb_s", [Dsh, 2], F32)
    xproj_T = sb("xproj_T_s", [Dsh, K, XD], BF16)
    dtw02_s = sb("dtw02_s", [2 * R, K * Dsh], BF16)
    dtw1_s = sb("dtw1_s", [R, K * Dsh], BF16)
    dtb_s = sb("dtb96_s", [K * Dsh, 1], F32)
    A_col = sb("A_col_s", [128, K * G], F32)
    Ddiag_s = sb("Ddiag_s", [Dsh, Dsh], BF16)
    red_s = sb("red_s", [128, G, Dsh], BF16)
    seldd_s = sb("seldd_s", [DG, 128], BF16)
    selst = sb("selst_s", [128, 2, 4 * B], BF16)
    lnwb4 = sb("lnwb4_s", [128, 2], F32)
    opw4 = sb("opw4_s", [128, C], BF16)
    opw_s = sb("opw_s", [Dsh, 2, C], BF16)

    def allreduce(in_ap, out_ap):
        if cfg.fake_cc:
            nc.sync.dma_start(out=out_ap, in_=in_ap)
        else:
            nc.gpsimd.collective_compute(
                "AllReduce", OP.add, replica_groups=groups,
                ins=[in_ap.opt()], outs=[out_ap.opt()])

    with tile.TileContext(nc) as tc:
        # ---- load weights ----
        for dst, srcw in [
            (w_in_ms, t["w_in_ms"]), (w_in_pan, t["w_in_pan"]),
            (convd[Dsh:2 * Dsh, 0], t["conv_d_ms"].rearrange(
                "p (x d) -> p x d", d=Dsh)),
            (convd[Dsh:2 * Dsh, 1], t["conv_d_pan"].rearrange(
                "p (x d) -> p x d", d=Dsh)),
            (convb[:, 0:1], t["conv_b_ms"]), (convb[:, 1:2], t["conv_b_pan"]),
            (xproj_T, t["xproj_T"].rearrange("p (k x) -> p k x", x=XD)),
            (dtw02_s, t["dtw02"]), (dtw1_s, t["dtw1"]), (dtb_s, t["dtb96"]),
            (A_col, t["A_col"]), (Ddiag_s, t["Ddiag"]),
            (red_s, t["red"].rearrange("p (g d) -> p g d", d=Dsh)),
            (seldd_s, t["seldd"]),
            (selst, t["selst"].rearrange("p (x m) -> p x m", m=4 * B)),
            (lnwb4, t["lnwb4"]), (opw4, t["opw4"]),
            (opw_s, t["opw"].rearrange("p (s c) -> p s c", c=C)),
        ]:
            nc.sync.dma_start(out=dst, in_=srcw)

        # ================= Phase F: in_proj + conv + silu =================
        with tc.tile_pool(name="f_ps", bufs=2, space="PSUM") as f_ps, \
             tc.tile_pool(name="f_cv", bufs=2, space="PSUM") as f_cv, \
             tc.tile_pool(name="f_src", bufs=3) as f_src, \
             tc.tile_pool(name="f_st", bufs=3) as f_st, \
             tc.tile_pool(name="f_xpad", bufs=2) as f_xpad:
            for b in range(B):
                xpads = []
                for s in range(2):
                    srcT = t["msf"] if s == 0 else t["panf"]
                    w_in = w_in_ms if s == 0 else w_in_pan
                    xpad = f_xpad.tile([2 * Dsh, Hp, Wp], BF16, tag=f"xpad{s}")
                    nc.vector.memset(xpad[Dsh:2 * Dsh], 0.0)
                    xpads.append(xpad)
                    for j in range(L // TOK):
                        mt = f_src.tile([C, TOK], BF16, tag="msrc")
                        nc.sync.dma_start(out=mt,
                                          in_=srcT[b, :, j * TOK:(j + 1) * TOK])
                        ps = f_ps.tile([2 * Dsh, TOK], F32, tag="fps")
                        mm(nc, ps, w_in, mt, start=True, stop=True)
                        rpc = TOK // W
                        nc.scalar.copy(
                            out=xpad[Dsh:2 * Dsh,
                                     1 + j * rpc:1 + (j + 1) * rpc, 1:1 + W],
                            in_=ps[Dsh:2 * Dsh, :].rearrange(
                                "p (r w) -> p r w", w=W))
                        zt = f_st.tile([Dsh, TOK], BF16, tag="zst")
                        nc.scalar.activation(out=zt, in_=ps[0:Dsh, :],
                                             func=AF.Silu)
                        nc.sync.dma_start(
                            out=zs_dram[:, 2 * b + s, j * TOK:(j + 1) * TOK],
                            in_=zt)
                # conv both streams per row-chunk, interleave in SBUF,
                # then one contiguous DMA write per chunk
                for j in range(H // CRW):
                    it2 = f_st.tile([Dsh, 2 * CRW * W], BF16, tag="it2")
                    for s in range(2):
                        cps = f_cv.tile([Dsh, CRW * W], F32, tag="cps")
                        for tap in range(9):
                            ky, kx = tap // 3, tap % 3
                            rhs = xpads[s][Dsh:2 * Dsh,
                                           ky + j * CRW: ky + (j + 1) * CRW,
                                           kx:kx + W]
                            nc.tensor.matmul(cps, convd[Dsh:2 * Dsh, s, tap, :],
                                             rhs,
                                             start=(tap == 0), stop=(tap == 8),
                                             skip_group_check=True)
                        ct = f_st.tile([Dsh, CRW * W], BF16, tag="cst")
                        nc.scalar.activation(out=ct, in_=cps, func=AF.Silu,
                                             bias=convb[:, s:s + 1])
                        nc.vector.tensor_copy(out=it2[:, s::2], in_=ct)
                    nc.sync.dma_start(
                        out=inter_dram[:, b, j * 2 * CRW * W:
                                       (j + 1) * 2 * CRW * W],
                        in_=it2)

        # ============ Phase X + AR + scan, shared PSUM budget ============
        with tc.tile_pool(name="x_ps", bufs=2, space="PSUM") as x_ps, \
             tc.tile_pool(name="s_dt_ps", bufs=2, space="PSUM") as s_dt_ps, \
             tc.tile_pool(name="s_nd_ps", bufs=2, space="PSUM") as s_nd_ps, \
             tc.tile_pool(name="s_y_ps", bufs=1, space="PSUM") as s_y_ps, \
             tc.tile_pool(name="x_ib", bufs=3) as x_ib, \
             tc.tile_pool(name="x_st", bufs=3) as x_st, \
             tc.tile_pool(name="s_io", bufs=2) as s_io, \
             tc.tile_pool(name="s_g8", bufs=1) as s_g8, \
             tc.tile_pool(name="s_dt", bufs=2) as s_dt, \
             tc.tile_pool(name="s_a", bufs=3) as s_a, \
             tc.tile_pool(name="s_b", bufs=3) as s_b, \
             tc.tile_pool(name="s_h", bufs=2) as s_h, \
             tc.tile_pool(name="s_hc", bufs=3) as s_hc, \
             tc.tile_pool(name="s_rep", bufs=3) as s_rep:

            def phase_x(b):
                for j in range(L2 // PC):
                    ibt = x_ib.tile([Dsh, PC], BF16, tag="xib")
                    nc.sync.dma_start(
                        out=ibt, in_=inter_dram[:, b, j * PC:(j + 1) * PC])
                    for k in range(K):
                        st = x_st.tile([XD, PC], BF16, tag="xst")
                        for h in range(PC // HMM):
                            ps = x_ps.tile([XD, HMM], F32, tag="xps")
                            nc.tensor.matmul(
                                ps, xproj_T[:, k, :],
                                ibt[:, h * HMM:(h + 1) * HMM],
                                start=True, stop=True, skip_group_check=True)
                            nc.scalar.copy(out=st[:, h * HMM:(h + 1) * HMM],
                                           in_=ps)
                        nc.sync.dma_start(
                            out=xdbl_part[b, k * XD:(k + 1) * XD,
                                          j * PC:(j + 1) * PC],
                            in_=st)

            # X(b0); AR(b0); X(b1); scan(b0); AR(b1); scan(b1) — the AR(b1)
            # trigger sits after scan(b0)'s Pool muls so the in-order Pool
            # queue doesn't stall on X(b1) completion before scanning b0.
            phase_x(0)
            allreduce(xdbl_part[0], xdbl_full[0])
            phase_x(1)

            h_prev = {}

            def phase_scan(b):
                for ch in range(NCH):
                    par = ch % 2
                    cs = slice(ch * Tc, (ch + 1) * Tc)
                    mcs = slice(L2 - (ch + 1) * Tc, L2 - ch * Tc)
                    xfb = xdbl_full[b]
                    xoff = b * (K * XD * L2)

                    ibc = s_io.tile([Dsh, Tc], BF16, tag="ibc")
                    nc.sync.dma_start(out=ibc, in_=inter_dram[:, b, cs])
                    ibmc = s_io.tile([Dsh, Tc], BF16, tag="ibmc")
                    nc.sync.dma_start(out=ibmc, in_=inter_dram[:, b, mcs])
                    # dt rows k0+k2 fwd, k1 mirror
                    stg16 = s_io.tile([2 * R, Tc], BF16, tag="stg16")
                    nc.sync.dma_start(
                        out=stg16,
                        in_=dram_ap(xdbl_full, xoff + ch * Tc,
                                    [[2 * XD * L2, 2], [L2, R], [1, Tc]]))
                    stg8 = s_io.tile([R, Tc], BF16, tag="stg8")
                    nc.sync.dma_start(
                        out=stg8, in_=xfb[XD:XD + R, mcs])
                    stg8R = s_io.tile([R, Tc], BF16, tag="stg8R")
                    nc.vector.tensor_copy(out=stg8R, in_=stg8[:, ::-1])
                    # k1 B/C rows (mirror) -> reversed -> DRAM bounce
                    bcF = s_io.tile([2 * N, Tc], BF16, tag="bcF")
                    nc.sync.dma_start(
                        out=bcF, in_=xfb[XD + R:XD + R + 2 * N, mcs])
                    bcR = s_io.tile([2 * N, Tc], BF16, tag="bcR")
                    nc.vector.tensor_copy(out=bcR, in_=bcF[:, ::-1])
                    nc.sync.dma_start(out=brc_dram[:, par, :], in_=bcR)
                    # fused dt: Exp then Ln(1+x) = softplus
                    e96 = s_dt.tile([K * Dsh, Tc], F32, tag="e96")
                    for h in range(Tc // HMM):
                        hs = slice(h * HMM, (h + 1) * HMM)
                        dtp = s_dt_ps.tile([K * Dsh, HMM], F32, tag="dtp")
                        nc.tensor.matmul(dtp, dtw02_s, stg16[:, hs],
                                         start=True, stop=False,
                                         skip_group_check=True)
                        nc.tensor.matmul(dtp, dtw1_s, stg8R[:, hs],
                                         start=False, stop=True,
                                         skip_group_check=True)
                        nc.scalar.activation(out=e96[:, hs], in_=dtp,
                                             func=AF.Exp,
                                             bias=dtb_s[:, 0:1], scale=1.0)
                    dt96 = s_dt.tile([K * Dsh, Tc], BF16, tag="dt96")
                    nc.scalar.activation(out=dt96, in_=e96, func=AF.Ln,
                                         bias=1.0)
                    # x3 = [ibc; rev(ibmc); ibc] then one fused mul
                    # (tensor_tensor requires same start partition on all
                    # operands; copies don't)
                    x3 = s_dt.tile([K * Dsh, Tc], BF16, tag="x3")
                    nc.vector.tensor_copy(out=x3[0:Dsh], in_=ibc)
                    nc.vector.tensor_copy(out=x3[Dsh:2 * Dsh],
                                          in_=ibmc[:, ::-1])
                    nc.vector.tensor_copy(out=x3[2 * Dsh:3 * Dsh], in_=ibc)
                    dtx96 = s_dt.tile([K * Dsh, Tc], BF16, tag="dtx96")
                    nc.vector.tensor_mul(dtx96, dt96, x3)
                    # bounce dt96 -> [8, 12*Tc] base-0 staging
                    nc.sync.dma_start(out=dtg_dram[par], in_=dt96)
                    g8 = s_g8.tile([DG, K * G, Tc], BF16, tag="g8")
                    nc.sync.dma_start(
                        out=g8,
                        in_=dram_ap(dtg_dram, par * (K * Dsh * Tc),
                                    [[Tc, DG], [DG * Tc, K * G], [1, Tc]]))
                    # B/C replicated loads
                    brep = {}
                    crep = {}
                    for k in (0, 2):
                        bk = s_rep.tile([128, Tc], BF16, tag=f"brep{k}")
                        nc.scalar.dma_start(
                            out=bk, in_=rep_dram(
                                xdbl_full,
                                xoff + (k * XD + R) * L2 + ch * Tc,
                                L2, N, DG, 1, Tc))
                        brep[k] = bk
                        ck = s_rep.tile([128, Tc], BF16, tag=f"crep{k}")
                        nc.scalar.dma_start(
                            out=ck, in_=rep_dram(
                                xdbl_full,
                                xoff + (k * XD + R + N) * L2 + ch * Tc,
                                L2, N, DG, 1, Tc))
                        crep[k] = ck
                    b1 = s_rep.tile([128, Tc], BF16, tag="brep1")
                    nc.scalar.dma_start(
                        out=b1, in_=rep_dram(brc_dram, par * Tc,
                                             2 * Tc, N, DG, 1, Tc))
                    brep[1] = b1
                    c1 = s_rep.tile([128, Tc], BF16, tag="crep1")
                    nc.scalar.dma_start(
                        out=c1, in_=rep_dram(brc_dram, (2 * N + par) * Tc,
                                             2 * Tc, N, DG, 1, Tc))
                    crep[1] = c1

                    y_ps = s_y_ps.tile([2 * Dsh, Tc], F32, tag="yps")
                    # D*x folded into the y02 rows (start of accumulation)
                    for h in range(Tc // HMM):
                        hs = slice(h * HMM, (h + 1) * HMM)
                        nc.tensor.matmul(y_ps[0:Dsh, hs], Ddiag_s,
                                         ibc[:, hs], start=True, stop=False,
                                         skip_group_check=True)

                    for k in range(K):
                        for g in range(G):
                            ci = k * G + g
                            a_t = s_a.tile([128, Tc], BF16, tag="a")
                            for h in range(Tc // HMM):
                                hs = slice(h * HMM, (h + 1) * HMM)
                                ndp = s_nd_ps.tile([128, HMM], F32, tag="ndp")
                                nc.tensor.matmul(ndp, seldd_s,
                                                 g8[:, ci, hs],
                                                 start=True, stop=True,
                                                 skip_group_check=True)
                                nc.scalar.activation(
                                    out=a_t[:, hs], in_=ndp, func=AF.Exp,
                                    scale=A_col[:, ci:ci + 1])
                            dtxr = s_rep.tile([128, Tc], BF16, tag="dxr")
                            dq = nc.sync if ci % 2 == 0 else nc.scalar
                            dq.dma_start(
                                out=dtxr,
                                in_=rep_sbuf(
                                    dtx96[k * Dsh + g * DG:
                                          k * Dsh + (g + 1) * DG, :], N))
                            b_t = s_b.tile([128, Tc], BF16, tag="bt")
                            nc.gpsimd.tensor_mul(b_t, dtxr, brep[k])
                            h_t = s_h.tile([128, Tc], BF16, tag=f"h{ci}")
                            key = (b, ci)
                            init = 0.0 if ch == 0 else \
                                h_prev[key][:, Tc - 1:Tc]
                            nc.vector.tensor_tensor_scan(
                                h_t, a_t, b_t, init, OP.mult, OP.add)
                            h_prev[key] = h_t
                            hc = s_hc.tile([128, Tc], BF16, tag="hc")
                            if ci in (0, 6):
                                nc.gpsimd.tensor_mul(hc, h_t, crep[k])
                            else:
                                nc.vector.tensor_mul(hc, h_t, crep[k])
                            ro = 0 if k != 1 else Dsh
                            last = (k == 2 and g == G - 1) if k != 1 \
                                else (g == G - 1)
                            for h in range(Tc // HMM):
                                hs = slice(h * HMM, (h + 1) * HMM)
                                nc.tensor.matmul(
                                    y_ps[ro:ro + Dsh, hs], red_s[:, g, :],
                                    hc[:, hs],
                                    start=(k == 1 and g == 0),
                                    stop=last, skip_group_check=True)
                    yev = s_io.tile([2 * Dsh, Tc], BF16, tag="yev")
                    nc.scalar.copy(out=yev, in_=y_ps)
                    nc.sync.dma_start(
                        out=dram_ap(yev_dram, b * 2 * L2 + ch * Tc,
                                    [[L2, 2], [B * 2 * L2, Dsh], [1, Tc]]),
                        in_=yev)

            phase_scan(0)
            allreduce(xdbl_part[1], xdbl_full[1])
            phase_scan(1)

        # ============ Phase L: LN stats + allreduce + apply + out_proj ======
        half = 2 * B * L
        with tc.tile_pool(name="l_ps", bufs=2, space="PSUM") as l_ps, \
             tc.tile_pool(name="l_one", bufs=1) as l_one, \
             tc.tile_pool(name="l_sq", bufs=3) as l_sq, \
             tc.tile_pool(name="l_stg", bufs=4) as l_stg:
            ydp = l_one.tile([128, L], BF16, tag="ydp")
            MC = 1024
            for b in range(B):
                for jj in range(L2 // MC):
                    yft = l_stg.tile([Dsh, MC], BF16, tag="yft")
                    nc.sync.dma_start(
                        out=yft, in_=yev_dram[:, b, 0, jj * MC:(jj + 1) * MC])
                    y1t = l_stg.tile([Dsh, MC], BF16, tag="y1t")
                    nc.sync.dma_start(
                        out=y1t,
                        in_=yev_dram[:, b, 1, L2 - (jj + 1) * MC:L2 - jj * MC])
                    ym = l_stg.tile([Dsh, MC], BF16, tag="ym")
                    nc.vector.tensor_add(ym, yft, y1t[:, ::-1])
                    tok = slice(jj * MC // 2, (jj + 1) * MC // 2)
                    nc.vector.tensor_copy(
                        out=ydp[(2 * b) * Dsh:(2 * b + 1) * Dsh, tok],
                        in_=ym[:, 0::2])
                    nc.vector.tensor_copy(
                        out=ydp[(2 * b + 1) * Dsh:(2 * b + 2) * Dsh, tok],
                        in_=ym[:, 1::2])
            for j in range(nLch):
                js = slice(j * TcL, (j + 1) * TcL)
                sqp = l_sq.tile([128, TcL], BF16, tag="sqp")
                nc.vector.tensor_mul(sqp, ydp[:, js], ydp[:, js])
                sp = l_ps.tile([4 * B, TcL], F32, tag="sps")
                mm(nc, sp, selst[:, 0, :], ydp[:, js], start=True, stop=False)
                mm(nc, sp, selst[:, 1, :], sqp, start=False, stop=True)
                stg = l_stg.tile([4 * B, TcL], F32, tag="sstg2")
                nc.scalar.copy(out=stg, in_=sp)
                nc.sync.dma_start(
                    out=stats_part.rearrange("a x l -> (a x) l")[:, js], in_=stg)
            allreduce(stats_part, stats_full)
            s1f = l_one.tile([64, SC], F32, tag="s1f")
            s2f = l_one.tile([64, SC], F32, tag="s2f")
            flat = stats_full.rearrange("a x l -> (a x l)")
            nc.sync.dma_start(
                out=s1f, in_=flat[0:half].rearrange("(p c) -> p c", p=64))
            nc.sync.dma_start(
                out=s2f, in_=flat[half:2 * half].rearrange("(p c) -> p c", p=64))
            mu_t = l_one.tile([64, SC], F32, tag="mu_t")
            var_t = l_one.tile([64, SC], F32, tag="var_t")
            musq = l_one.tile([64, SC], F32, tag="musq")
            eps_t = l_one.tile([64, 1], F32, tag="eps_t")
            nc.vector.memset(eps_t, 1e-5)
            nc.vector.tensor_scalar_mul(mu_t, s1f, 1.0 / D)
            nc.vector.tensor_scalar_mul(var_t, s2f, 1.0 / D)
            nc.vector.tensor_mul(musq, mu_t, mu_t)
            nc.vector.tensor_sub(var_t, var_t, musq)
            nc.scalar.activation(out=var_t, in_=var_t, func=AF.Sqrt, bias=eps_t)
            nc.vector.reciprocal(out=s1f, in_=var_t)          # alpha
            nc.vector.tensor_mul(s2f, mu_t, s1f)
            nc.vector.tensor_scalar_mul(s2f, s2f, -1.0)       # beta
            nc.sync.dma_start(
                out=ab_dram.rearrange("a x l -> (a x l)")[0:half].rearrange(
                    "(p c) -> p c", p=64), in_=s1f)
            nc.sync.dma_start(
                out=ab_dram.rearrange("a x l -> (a x l)")[half:2 * half].rearrange(
                    "(p c) -> p c", p=64), in_=s2f)

            # apply LN + gate + out_proj; stream s=0 fully, RS, then s=1, RS
            with tc.tile_pool(name="l_rep", bufs=3) as l_rep, \
                 tc.tile_pool(name="l_t", bufs=3) as l_t, \
                 tc.tile_pool(name="l_fg", bufs=1) as l_fg, \
                 tc.tile_pool(name="o_st", bufs=2) as o_st, \
                 tc.tile_pool(name="o_ps", bufs=2, space="PSUM") as o_ps:
                fgs = []
                for j in range(nLch):
                    js = slice(j * TcL, (j + 1) * TcL)
                    zcp = l_rep.tile([128, TcL], BF16, tag="zcp")
                    nc.sync.dma_start(
                        out=zcp,
                        in_=bass.AP(tensor=zs_dram.tensor,
                                    offset=zs_dram.offset + j * TcL,
                                    ap=[[L, 2 * B], [2 * B * L, Dsh],
                                        [1, TcL]]))
                    zwp = l_t.tile([128, TcL], BF16, tag="zwp")
                    bzp = l_t.tile([128, TcL], BF16, tag="bzp")
                    nc.vector.tensor_scalar_mul(zwp, zcp, lnwb4[:, 0:1])
                    nc.vector.tensor_scalar_mul(bzp, zcp, lnwb4[:, 1:2])
                    arp = l_rep.tile([128, TcL], F32, tag="arp")
                    brp = l_rep.tile([128, TcL], F32, tag="brp")
                    nc.sync.dma_start(
                        out=arp,
                        in_=bass.AP(tensor=ab_dram.tensor,
                                    offset=ab_dram.offset + j * TcL,
                                    ap=[[L, 2 * B], [0, Dsh], [1, TcL]]))
                    nc.scalar.dma_start(
                        out=brp,
                        in_=bass.AP(tensor=ab_dram.tensor,
                                    offset=ab_dram.offset + half + j * TcL,
                                    ap=[[L, 2 * B], [0, Dsh], [1, TcL]]))
                    t1 = l_t.tile([128, TcL], BF16, tag="t1")
                    nc.vector.tensor_mul(t1, ydp[:, js], arp)
                    t2 = l_t.tile([128, TcL], BF16, tag="t2")
                    nc.vector.tensor_add(t2, t1, brp)
                    t3 = l_t.tile([128, TcL], BF16, tag="t3")
                    nc.vector.tensor_mul(t3, t2, zwp)
                    fgp = l_fg.tile([128, TcL], BF16, tag=f"fgp{j}")
                    nc.vector.tensor_add(fgp, t3, bzp)
                    fgs.append(fgp)
                for s in range(2):
                    for j in range(nLch):
                        js = slice(j * TcL, (j + 1) * TcL)
                        fgp = fgs[j]
                        for b in range(B):
                            bs = b * 2 + s
                            qs = slice(bs * Dsh, (bs + 1) * Dsh)
                            if bs * Dsh in (0, 32, 64):
                                lhs = opw4[qs, :]
                                rhs = fgp[qs, :]
                            else:
                                stq = o_st.tile([Dsh, TcL], BF16, tag="stq")
                                nc.sync.dma_start(out=stq, in_=fgp[qs, :])
                                lhs = opw_s[:, s, :]
                                rhs = stq
                            ops = o_ps.tile([C, TcL], F32, tag="ops")
                            mm(nc, ops, lhs, rhs, start=True, stop=True)
                            ost = o_st.tile([C, TcL], F32, tag="ost")
                            nc.scalar.copy(out=ost, in_=ops)
                            nc.sync.dma_start(out=outp_part[s, b, :, js],
                                              in_=ost)
                    # ReduceScatter this stream's half
                    hflat = flat_out // 2
                    hchunk = chunk_o // 2
                    pflat = outp_part.rearrange("s b c l -> (s b c l)")
                    if cfg.fake_cc:
                        nc.sync.dma_start(
                            out=out_chunk[s * hchunk:(s + 1) * hchunk],
                            in_=pflat[s * hflat:s * hflat + hchunk])
                    else:
                        nc.gpsimd.collective_compute(
                            "ReduceScatter", OP.add, replica_groups=groups,
                            ins=[pflat[s * hflat:(s + 1) * hflat].opt()],
                            outs=[outp_rs[s * hchunk:(s + 1) * hchunk].opt()])
                        nc.sync.dma_start(
                            out=out_chunk[s * hchunk:(s + 1) * hchunk],
                            in_=outp_rs[s * hchunk:(s + 1) * hchunk])

    nc.compile()
    return nc


# ---------------------------------------------------------------------------
# public entry point
# ---------------------------------------------------------------------------

_CACHE = {}


def _get_nc(cfg: Cfg):
    if cfg not in _CACHE:
        _CACHE[cfg] = build_nc(cfg)
    return _CACHE[cfg]


def kernel(**inputs):
    cfg = CFG
    nc = _get_nc(cfg)
    in_maps = host_prep(cfg, inputs)
    res = bass_utils.run_bass_kernel_spmd(
        nc, in_maps, core_ids=list(range(cfg.n_cores)))
    return assemble_outputs(cfg, res.results)


def assemble_outputs(cfg, results):
    """Each core returns its ReduceScatter chunks: [ms-chunk | pan-chunk]."""
    B, C, L = cfg.B, cfg.C, cfg.L
    hflat = B * C * L
    hchunk = hflat // cfg.n_cores
    ms_flat = np.zeros(hflat, np.float32)
    pan_flat = np.zeros(hflat, np.float32)
    for r in range(cfg.n_cores):
        ck = np.asarray(results[r]["out_chunk"], np.float32)
        ms_flat[r * hchunk:(r + 1) * hchunk] = ck[0:hchunk]
        pan_flat[r * hchunk:(r + 1) * hchunk] = ck[hchunk:2 * hchunk]
    out_ms = ms_flat.reshape(B, C, cfg.H, cfg.W)
    out_pan = pan_flat.reshape(B, C, cfg.H, cfg.W)
    return (out_ms, out_pan)


# revision 13
# speedup vs baseline: 2.8974x; 1.3092x over previous
"""Trainium2 Bass kernel for nn_DSSM_57629871178390 (dual-stream Mamba/DSSM block).

Sharding: d_inner=256 split 8 ways across cores (Dsh=32 channels each). The
selective scan runs on the DVE via tensor_tensor_scan with 128-partition
tiles laid out as (dd=8 d-values x n=16 states); K=3 x G=4 tiles per (b,chunk).

v2 restructure vs baseline:
- softplus(dt) via Exp+Ln (one ACT table with the scan's Exp -> no table loads)
- dt for all 3 scan routes fused into one [96,Tc] matmul+ACT chain
- ndt replication via PE matmul from a [8,12*Tc] DRAM-bounced staging tile
- b_t muls on the Pool engine (GpSimd), hc muls mostly on DVE
- inter stored interleaved in DRAM (no per-load de-interleave copies)
- per-batch AllReduce of x_dbl, overlapped with the other batch's scan
- ReduceScatter split per stream (ms/pan) to overlap with out_proj
- scan carry chained via previous h tile's last column (no copy ops)

Self-contained: hardcodes all shapes; imports numpy + concourse (available
at /opt/trn_rl_repo inside the container).
"""

import sys
from dataclasses import dataclass

import numpy as np

if "/opt/trn_rl_repo" not in sys.path:
    sys.path.insert(0, "/opt/trn_rl_repo")

import ml_dtypes  # noqa: E402

import concourse.bass as bass  # noqa: E402
import concourse.bacc as bacc  # noqa: E402
import concourse.tile as tile  # noqa: E402
import concourse.mybir as mybir  # noqa: E402
from concourse import bass_utils  # noqa: E402

F32 = mybir.dt.float32
BF16 = mybir.dt.bfloat16
AF = mybir.ActivationFunctionType
OP = mybir.AluOpType
NPBF16 = ml_dtypes.bfloat16


@dataclass(frozen=True)
class Cfg:
    B: int = 2
    C: int = 128
    H: int = 64
    W: int = 64
    D: int = 256          # d_inner
    N: int = 16           # d_state
    R: int = 8            # dt_rank
    K: int = 3
    n_cores: int = 8
    fake_cc: bool = False   # replace collectives with plain DMA (profiling)

    @property
    def L(self):
        return self.H * self.W

    @property
    def L2(self):
        return 2 * self.L

    @property
    def Dsh(self):
        return self.D // self.n_cores

    @property
    def DG(self):
        return 128 // self.N       # d-values per 128-partition tile (8)

    @property
    def G(self):
        return self.Dsh // self.DG  # d-groups per core (4)

    @property
    def Tc(self):
        return 1024                 # scan chunk

    @property
    def TOK(self):
        return min(512, self.L)     # in_proj token chunk

    @property
    def CR(self):
        return max(1, min(512 // self.W, self.H))  # conv rows per chunk

    @property
    def TcL(self):
        return min(512, self.L)     # LN/out_proj chunk


CFG = Cfg()
MMF = 512  # max moving free dim per matmul


# ---------------------------------------------------------------------------
# Host-side preparation of per-core input maps
# ---------------------------------------------------------------------------

def host_prep(cfg: Cfg, inputs: dict) -> list:
    B, C, H, W = cfg.B, cfg.C, cfg.H, cfg.W
    D, N, R, K = cfg.D, cfg.N, cfg.R, cfg.K
    Dsh, G, DG, L = cfg.Dsh, cfg.G, cfg.DG, cfg.L

    ms = np.asarray(inputs["ms"], np.float32).reshape(B, C, L)
    pan = np.asarray(inputs["pan"], np.float32).reshape(B, C, L)
    w_ms = np.asarray(inputs["in_proj_ms_w"], np.float32)
    w_pan = np.asarray(inputs["in_proj_pan_w"], np.float32)
    cw_ms = np.asarray(inputs["conv_ms_w"], np.float32)
    cb_ms = np.asarray(inputs["conv_ms_b"], np.float32)
    cw_pan = np.asarray(inputs["conv_pan_w"], np.float32)
    cb_pan = np.asarray(inputs["conv_pan_b"], np.float32)
    xpw = np.asarray(inputs["x_proj_weight"], np.float32)
    dtw = np.asarray(inputs["dt_projs_weight"], np.float32)
    dtb = np.asarray(inputs["dt_projs_bias"], np.float32)
    A_logs = np.asarray(inputs["A_logs"], np.float32)
    Ds = np.asarray(inputs["Ds"], np.float32)
    lnw_v = np.asarray(inputs["out_norm_vis_w"], np.float32)
    lnb_v = np.asarray(inputs["out_norm_vis_b"], np.float32)
    lnw_i = np.asarray(inputs["out_norm_inf_w"], np.float32)
    lnb_i = np.asarray(inputs["out_norm_inf_b"], np.float32)
    opw_ms = np.asarray(inputs["out_proj_ms_w"], np.float32)
    opw_pan = np.asarray(inputs["out_proj_pan_w"], np.float32)

    A = np.exp(A_logs).reshape(K, D, N)
    Dsum = Ds.reshape(K, D).sum(0)

    # partition layout of scan tiles: p = dd*N + n
    red = np.zeros((128, G * Dsh), NPBF16)
    for g in range(G):
        for p in range(128):
            red[p, g * Dsh + DG * g + (p // N)] = 1.0

    selst_h = np.zeros((128, 2, 4 * B), NPBF16)
    for pi in range(128):
        selst_h[pi, 0, pi // Dsh] = 1.0          # S1 rows
        selst_h[pi, 1, 2 * B + pi // Dsh] = 1.0  # S2 rows
    seldd = np.zeros((DG, 128), NPBF16)
    for pi in range(128):
        seldd[pi // N, pi] = 1.0

    shared = {
        "seldd": seldd,
        "selst": selst_h.reshape(128, 2 * 4 * B),
        "msf": ms.astype(NPBF16),
        "panf": pan.astype(NPBF16),
        "red": red,
    }

    maps = []
    for c in range(cfg.n_cores):
        dsl = slice(Dsh * c, Dsh * (c + 1))
        m = dict(shared)
        # cols 0:Dsh = z rows (W[D+dsl]), cols Dsh:2Dsh = x rows (W[dsl])
        m["w_in_ms"] = np.ascontiguousarray(
            np.concatenate([w_ms[D + Dsh * c: D + Dsh * (c + 1)].T,
                            w_ms[dsl].T], 1)).astype(NPBF16)
        m["w_in_pan"] = np.ascontiguousarray(
            np.concatenate([w_pan[D + Dsh * c: D + Dsh * (c + 1)].T,
                            w_pan[dsl].T], 1)).astype(NPBF16)
        cd_ms = np.zeros((Dsh, 9 * Dsh), NPBF16)
        cd_pan = np.zeros((Dsh, 9 * Dsh), NPBF16)
        for t in range(9):
            ky, kx = t // 3, t % 3
            for i in range(Dsh):
                cd_ms[i, t * Dsh + i] = cw_ms[Dsh * c + i, 0, ky, kx]
                cd_pan[i, t * Dsh + i] = cw_pan[Dsh * c + i, 0, ky, kx]
        m["conv_d_ms"] = cd_ms
        m["conv_d_pan"] = cd_pan
        m["conv_b_ms"] = cb_ms[dsl].reshape(Dsh, 1).astype(np.float32)
        m["conv_b_pan"] = cb_pan[dsl].reshape(Dsh, 1).astype(np.float32)
        m["xproj_T"] = np.ascontiguousarray(
            xpw[:, :, dsl].transpose(2, 0, 1).reshape(Dsh, K * (R + 2 * N))
        ).astype(NPBF16)
        # fused dt projection: out partition = k*Dsh + dl
        dtw02 = np.zeros((2 * R, 3 * Dsh), NPBF16)
        dtw1 = np.zeros((R, 3 * Dsh), NPBF16)
        for r in range(R):
            for dl in range(Dsh):
                dtw02[r, dl] = dtw[0, Dsh * c + dl, r]
                dtw02[R + r, 2 * Dsh + dl] = dtw[2, Dsh * c + dl, r]
                dtw1[r, Dsh + dl] = dtw[1, Dsh * c + dl, r]
        m["dtw02"] = dtw02
        m["dtw1"] = dtw1
        m["dtb96"] = np.ascontiguousarray(
            dtb[:, dsl].reshape(K * Dsh, 1)).astype(np.float32)
        acol = np.zeros((128, K * G), np.float32)
        for k in range(K):
            for g in range(G):
                for p in range(128):
                    dd, n = p // N, p % N
                    acol[p, k * G + g] = -A[k, Dsh * c + DG * g + dd, n]
        m["A_col"] = acol
        Ddiag = np.zeros((Dsh, Dsh), NPBF16)
        for i in range(Dsh):
            Ddiag[i, i] = Dsum[Dsh * c + i]
        m["Ddiag"] = Ddiag
        lw4 = np.zeros((128, 2), np.float32)
        for bs in range(2 * B):
            sle = bs % 2
            lw4[bs * Dsh:(bs + 1) * Dsh, 0] = (lnw_v if sle == 0 else lnw_i)[dsl]
            lw4[bs * Dsh:(bs + 1) * Dsh, 1] = (lnb_v if sle == 0 else lnb_i)[dsl]
        m["lnwb4"] = lw4
        o4 = np.zeros((128, C), NPBF16)
        for bs in range(2 * B):
            sle = bs % 2
            o4[bs * Dsh:(bs + 1) * Dsh] = (
                (opw_ms if sle == 0 else opw_pan)[:, dsl].T).astype(NPBF16)
        m["opw4"] = o4
        m["opw"] = np.ascontiguousarray(
            np.concatenate([opw_ms[:, dsl].T, opw_pan[:, dsl].T], 1)
        ).astype(NPBF16)  # (Dsh, 2C)
        maps.append(m)
    return maps


# ---------------------------------------------------------------------------
# AP helpers
# ---------------------------------------------------------------------------

def rep_sbuf(ap2d, n_inner):
    """SBUF [P, F] slice -> src AP for dest [P*n_inner, F] with each source
    partition repeated n_inner times (dest p = i*n_inner + j reads src i)."""
    pairs = [list(x) for x in ap2d.ap]
    assert len(pairs) == 2
    return bass.AP(tensor=ap2d.tensor, offset=ap2d.offset,
                   ap=[pairs[0], [0, n_inner], pairs[1]])


def rep_dram(tensor_ap, offset, row_stride, n_rows, n_rep, fstep, fcount):
    """DRAM source AP replicating a [n_rows, fcount] block so that dest
    partition p = j*n_rows + r reads row r (rep index j outer)."""
    return bass.AP(tensor=tensor_ap.tensor, offset=tensor_ap.offset + offset,
                   ap=[[0, n_rep], [row_stride, n_rows], [fstep, fcount]])


def dram_ap(tensor_ap, offset, pairs):
    return bass.AP(tensor=tensor_ap.tensor, offset=tensor_ap.offset + offset,
                   ap=pairs)


def mm(nc, out_ps, lhsT, rhs, start, stop, maxf=MMF):
    """Matmul with moving-free-dim splitting. out/rhs 2D [P, F]."""
    F = rhs.shape[-1]
    if F <= maxf:
        nc.tensor.matmul(out_ps, lhsT, rhs, start=start, stop=stop,
                         skip_group_check=True)
        return
    assert F % maxf == 0
    for i in range(F // maxf):
        nc.tensor.matmul(out_ps[:, i * maxf:(i + 1) * maxf], lhsT,
                         rhs[:, i * maxf:(i + 1) * maxf], start=start, stop=stop,
                         skip_group_check=True)


# ---------------------------------------------------------------------------
# Bass program builder
# ---------------------------------------------------------------------------

def build_nc(cfg: Cfg):
    B, C, H, W = cfg.B, cfg.C, cfg.H, cfg.W
    D, N, R, K = cfg.D, cfg.N, cfg.R, cfg.K
    Dsh, G, DG = cfg.Dsh, cfg.G, cfg.DG
    L, L2, Tc = cfg.L, cfg.L2, cfg.Tc
    TOK, CRW, TcL = cfg.TOK, cfg.CR, cfg.TcL
    NCH = L2 // Tc               # scan chunks per batch (16)
    PC = 1024                    # x_dbl phase chunk
    XD = R + 2 * N               # 40
    Hp, Wp = H + 2, W + 2
    groups = [list(range(cfg.n_cores))]
    nLch = L // TcL
    SC = 2 * B * L // 64         # stats cols for [64, SC] view
    HMM = 512

    nc = bacc.Bacc("TRN2", target_bir_lowering=False, debug=False,
                   enable_asserts=False, num_devices=cfg.n_cores)

    t = {}

    def inp(name, shape, dt):
        t[name] = nc.dram_tensor(name, shape, dt, kind="ExternalInput").ap()

    inp("msf", [B, C, L], BF16)
    inp("panf", [B, C, L], BF16)
    inp("w_in_ms", [C, 2 * Dsh], BF16)    # cols 0:Dsh z-rows, Dsh:2Dsh x-rows
    inp("w_in_pan", [C, 2 * Dsh], BF16)
    inp("conv_d_ms", [Dsh, 9 * Dsh], BF16)
    inp("conv_d_pan", [Dsh, 9 * Dsh], BF16)
    inp("conv_b_ms", [Dsh, 1], F32)
    inp("conv_b_pan", [Dsh, 1], F32)
    inp("xproj_T", [Dsh, K * XD], BF16)
    inp("dtw02", [2 * R, K * Dsh], BF16)
    inp("dtw1", [R, K * Dsh], BF16)
    inp("dtb96", [K * Dsh, 1], F32)
    inp("A_col", [128, K * G], F32)       # NEGATIVE -exp(A_logs)
    inp("Ddiag", [Dsh, Dsh], BF16)
    inp("red", [128, G * Dsh], BF16)
    inp("seldd", [DG, 128], BF16)
    inp("selst", [128, 2 * 4 * B], BF16)
    inp("lnwb4", [128, 2], F32)
    inp("opw4", [128, C], BF16)
    inp("opw", [Dsh, 2 * C], BF16)

    flat_out = 2 * B * C * L
    chunk_o = flat_out // cfg.n_cores
    out_chunk = nc.dram_tensor("out_chunk", [chunk_o], F32,
                               kind="ExternalOutput").ap()

    shsp = "Shared" if cfg.n_cores > 4 else "Local"
    inter_dram = nc.dram_tensor("inter_dram", [Dsh, B, L2], BF16,
                                kind="Internal").ap()
    zs_dram = nc.dram_tensor("zs_dram", [Dsh, 2 * B, L], BF16,
                             kind="Internal").ap()
    xdbl_part = nc.dram_tensor("xdbl_part", [B, K * XD, L2], BF16,
                               kind="Internal").ap()
    xdbl_full = nc.dram_tensor("xdbl_full", [B, K * XD, L2], BF16,
                               kind="Internal", addr_space=shsp).ap()
    dtg_dram = nc.dram_tensor("dtg_dram", [2, K * Dsh, Tc], BF16,
                              kind="Internal").ap()
    brc_dram = nc.dram_tensor("brc_dram", [2 * N, 2, Tc], BF16,
                              kind="Internal").ap()
    yev_dram = nc.dram_tensor("yev_dram", [Dsh, B, 2, L2], BF16,
                              kind="Internal").ap()
    stats_part = nc.dram_tensor("stats_part", [2, 2 * B, L], F32,
                                kind="Internal").ap()
    stats_full = nc.dram_tensor("stats_full", [2, 2 * B, L], F32,
                                kind="Internal", addr_space=shsp).ap()
    ab_dram = nc.dram_tensor("ab_dram", [2, 2 * B, L], F32,
                             kind="Internal").ap()
    outp_part = nc.dram_tensor("outp_part", [2, B, C, L], F32,
                               kind="Internal").ap()
    outp_rs = nc.dram_tensor("outp_rs", [2 * B * C * L // cfg.n_cores], F32,
                             kind="Internal").ap()

    # persistent SBUF weights
    def sb(name, shape, dt):
        return nc.alloc_sbuf_tensor(name, shape, dt).ap()

    w_in_ms = sb("w_in_ms_s", [C, 2 * Dsh], BF16)
    w_in_pan = sb("w_in_pan_s", [C, 2 * Dsh], BF16)
    convd = sb("convd_s", [2 * Dsh, 2, 9, Dsh], BF16)  # diag at rows Dsh:2Dsh
    convb = sb("convb_s", [Dsh, 2], F32)
    xproj_T = sb("xproj_T_s", [Dsh, K, XD], BF16)
    dtw02_s = sb("dtw02_s", [2 * R, K * Dsh], BF16)
    dtw1_s = sb("dtw1_s", [R, K * Dsh], BF16)
    dtb_s = sb("dtb96_s", [K * Dsh, 1], F32)
    A_col = sb("A_col_s", [128, K * G], F32)
    Ddiag_s = sb("Ddiag_s", [Dsh, Dsh], BF16)
    red_s = sb("red_s", [128, G, Dsh], BF16)
    seldd_s = sb("seldd_s", [DG, 128], BF16)
    selst = sb("selst_s", [128, 2, 4 * B], BF16)
    lnwb4 = sb("lnwb4_s", [128, 2], F32)
    opw4 = sb("opw4_s", [128, C], BF16)
    opw_s = sb("opw_s", [Dsh, 2, C], BF16)

    def allreduce(in_ap, out_ap):
        if cfg.fake_cc:
            nc.sync.dma_start(out=out_ap, in_=in_ap)
        else:
            nc.gpsimd.collective_compute(
                "AllReduce", OP.add, replica_groups=groups,
                ins=[in_ap.opt()], outs=[out_ap.opt()])

    with tile.TileContext(nc) as tc:
        # ---- load weights ----
        for dst, srcw in [
            (w_in_ms, t["w_in_ms"]), (w_in_pan, t["w_in_pan"]),
            (convd[Dsh:2 * Dsh, 0], t["conv_d_ms"].rearrange(
                "p (x d) -> p x d", d=Dsh)),
            (convd[Dsh:2 * Dsh, 1], t["conv_d_pan"].rearrange(
                "p (x d) -> p x d", d=Dsh)),
            (convb[:, 0:1], t["conv_b_ms"]), (convb[:, 1:2], t["conv_b_pan"]),
            (xproj_T, t["xproj_T"].rearrange("p (k x) -> p k x", x=XD)),
            (dtw02_s, t["dtw02"]), (dtw1_s, t["dtw1"]), (dtb_s, t["dtb96"]),
            (A_col, t["A_col"]), (Ddiag_s, t["Ddiag"]),
            (red_s, t["red"].rearrange("p (g d) -> p g d", d=Dsh)),
            (seldd_s, t["seldd"]),
            (selst, t["selst"].rearrange("p (x m) -> p x m", m=4 * B)),
            (lnwb4, t["lnwb4"]), (opw4, t["opw4"]),
            (opw_s, t["opw"].rearrange("p (s c) -> p s c", c=C)),
        ]:
            nc.sync.dma_start(out=dst, in_=srcw)

        # ================= Phase F: in_proj + conv + silu =================
        with tc.tile_pool(name="f_ps", bufs=2, space="PSUM") as f_ps, \
             tc.tile_pool(name="f_cv", bufs=2, space="PSUM") as f_cv, \
             tc.tile_pool(name="f_src", bufs=3) as f_src, \
             tc.tile_pool(name="f_st", bufs=3) as f_st, \
             tc.tile_pool(name="f_xpad", bufs=2) as f_xpad:
            for b in range(B):
                xpads = []
                for s in range(2):
                    srcT = t["msf"] if s == 0 else t["panf"]
                    w_in = w_in_ms if s == 0 else w_in_pan
                    xpad = f_xpad.tile([2 * Dsh, Hp, Wp], BF16, tag=f"xpad{s}")
                    nc.vector.memset(xpad[Dsh:2 * Dsh], 0.0)
                    xpads.append(xpad)
                    for j in range(L // TOK):
                        mt = f_src.tile([C, TOK], BF16, tag="msrc")
                        nc.sync.dma_start(out=mt,
                                          in_=srcT[b, :, j * TOK:(j + 1) * TOK])
                        ps = f_ps.tile([2 * Dsh, TOK], F32, tag="fps")
                        mm(nc, ps, w_in, mt, start=True, stop=True)
                        rpc = TOK // W
                        nc.scalar.copy(
                            out=xpad[Dsh:2 * Dsh,
                                     1 + j * rpc:1 + (j + 1) * rpc, 1:1 + W],
                            in_=ps[Dsh:2 * Dsh, :].rearrange(
                                "p (r w) -> p r w", w=W))
                        zt = f_st.tile([Dsh, TOK], BF16, tag="zst")
                        nc.scalar.activation(out=zt, in_=ps[0:Dsh, :],
                                             func=AF.Silu)
                        nc.sync.dma_start(
                            out=zs_dram[:, 2 * b + s, j * TOK:(j + 1) * TOK],
                            in_=zt)
                # conv both streams per row-chunk, interleave in SBUF,
                # then one contiguous DMA write per chunk
                for j in range(H // CRW):
                    it2 = f_st.tile([Dsh, 2 * CRW * W], BF16, tag="it2")
                    for s in range(2):
                        cps = f_cv.tile([Dsh, CRW * W], F32, tag="cps")
                        for tap in range(9):
                            ky, kx = tap // 3, tap % 3
                            rhs = xpads[s][Dsh:2 * Dsh,
                                           ky + j * CRW: ky + (j + 1) * CRW,
                                           kx:kx + W]
                            nc.tensor.matmul(cps, convd[Dsh:2 * Dsh, s, tap, :],
                                             rhs,
                                             start=(tap == 0), stop=(tap == 8),
                                             skip_group_check=True)
                        ct = f_st.tile([Dsh, CRW * W], BF16, tag="cst")
                        nc.scalar.activation(out=ct, in_=cps, func=AF.Silu,
                                             bias=convb[:, s:s + 1])
                        nc.vector.tensor_copy(out=it2[:, s::2], in_=ct)
                    nc.sync.dma_start(
                        out=inter_dram[:, b, j * 2 * CRW * W:
                                       (j + 1) * 2 * CRW * W],
                        in_=it2)

        # ============ Phase X + AR + scan, shared PSUM budget ============
        with tc.tile_pool(name="x_ps", bufs=2, space="PSUM") as x_ps, \
             tc.tile_pool(name="s_dt_ps", bufs=2, space="PSUM") as s_dt_ps, \
             tc.tile_pool(name="s_nd_ps", bufs=2, space="PSUM") as s_nd_ps, \
             tc.tile_pool(name="s_y_ps", bufs=1, space="PSUM") as s_y_ps, \
             tc.tile_pool(name="x_ib", bufs=3) as x_ib, \
             tc.tile_pool(name="x_st", bufs=3) as x_st, \
             tc.tile_pool(name="s_io", bufs=2) as s_io, \
             tc.tile_pool(name="s_g8", bufs=1) as s_g8, \
             tc.tile_pool(name="s_dt", bufs=2) as s_dt, \
             tc.tile_pool(name="s_a", bufs=3) as s_a, \
             tc.tile_pool(name="s_b", bufs=3) as s_b, \
             tc.tile_pool(name="s_h", bufs=2) as s_h, \
             tc.tile_pool(name="s_hc", bufs=3) as s_hc, \
             tc.tile_pool(name="s_rep", bufs=3) as s_rep:

            def phase_x(b):
                for j in range(L2 // PC):
                    ibt = x_ib.tile([Dsh, PC], BF16, tag="xib")
                    nc.sync.dma_start(
                        out=ibt, in_=inter_dram[:, b, j * PC:(j + 1) * PC])
                    for k in range(K):
                        st = x_st.tile([XD, PC], BF16, tag="xst")
                        for h in range(PC // HMM):
                            ps = x_ps.tile([XD, HMM], F32, tag="xps")
                            nc.tensor.matmul(
                                ps, xproj_T[:, k, :],
                                ibt[:, h * HMM:(h + 1) * HMM],
                                start=True, stop=True, skip_group_check=True)
                            nc.scalar.copy(out=st[:, h * HMM:(h + 1) * HMM],
                                           in_=ps)
                        nc.sync.dma_start(
                            out=xdbl_part[b, k * XD:(k + 1) * XD,
                                          j * PC:(j + 1) * PC],
                            in_=st)

            # X(b0); AR(b0); X(b1); scan(b0); AR(b1); scan(b1) — the AR(b1)
            # trigger sits after scan(b0)'s Pool muls so the in-order Pool
            # queue doesn't stall on X(b1) completion before scanning b0.
            phase_x(0)
            allreduce(xdbl_part[0], xdbl_full[0])
            phase_x(1)

            h_prev = {}

            def phase_scan(b):
                for ch in range(NCH):
                    par = ch % 2
                    cs = slice(ch * Tc, (ch + 1) * Tc)
                    mcs = slice(L2 - (ch + 1) * Tc, L2 - ch * Tc)
                    xfb = xdbl_full[b]
                    xoff = b * (K * XD * L2)

                    ibc = s_io.tile([Dsh, Tc], BF16, tag="ibc")
                    nc.sync.dma_start(out=ibc, in_=inter_dram[:, b, cs])
                    ibmc = s_io.tile([Dsh, Tc], BF16, tag="ibmc")
                    nc.sync.dma_start(out=ibmc, in_=inter_dram[:, b, mcs])
                    # dt rows k0+k2 fwd, k1 mirror
                    stg16 = s_io.tile([2 * R, Tc], BF16, tag="stg16")
                    nc.sync.dma_start(
                        out=stg16,
                        in_=dram_ap(xdbl_full, xoff + ch * Tc,
                                    [[2 * XD * L2, 2], [L2, R], [1, Tc]]))
                    stg8 = s_io.tile([R, Tc], BF16, tag="stg8")
                    nc.sync.dma_start(
                        out=stg8, in_=xfb[XD:XD + R, mcs])
                    stg8R = s_io.tile([R, Tc], BF16, tag="stg8R")
                    nc.vector.tensor_copy(out=stg8R, in_=stg8[:, ::-1])
                    # k1 B/C rows (mirror) -> reversed -> DRAM bounce
                    bcF = s_io.tile([2 * N, Tc], BF16, tag="bcF")
                    nc.sync.dma_start(
                        out=bcF, in_=xfb[XD + R:XD + R + 2 * N, mcs])
                    bcR = s_io.tile([2 * N, Tc], BF16, tag="bcR")
                    nc.vector.tensor_copy(out=bcR, in_=bcF[:, ::-1])
                    nc.sync.dma_start(out=brc_dram[:, par, :], in_=bcR)
                    # fused dt: Exp then Ln(1+x) = softplus
                    e96 = s_dt.tile([K * Dsh, Tc], F32, tag="e96")
                    for h in range(Tc // HMM):
                        hs = slice(h * HMM, (h + 1) * HMM)
                        dtp = s_dt_ps.tile([K * Dsh, HMM], F32, tag="dtp")
                        nc.tensor.matmul(dtp, dtw02_s, stg16[:, hs],
                                         start=True, stop=False,
                                         skip_group_check=True)
                        nc.tensor.matmul(dtp, dtw1_s, stg8R[:, hs],
                                         start=False, stop=True,
                                         skip_group_check=True)
                        nc.scalar.activation(out=e96[:, hs], in_=dtp,
                                             func=AF.Exp,
                                             bias=dtb_s[:, 0:1], scale=1.0)
                    dt96 = s_dt.tile([K * Dsh, Tc], BF16, tag="dt96")
                    nc.scalar.activation(out=dt96, in_=e96, func=AF.Ln,
                                         bias=1.0)
                    # x3 = [ibc; rev(ibmc); ibc] then one fused mul
                    # (tensor_tensor requires same start partition on all
                    # operands; copies don't)
                    x3 = s_dt.tile([K * Dsh, Tc], BF16, tag="x3")
                    nc.vector.tensor_copy(out=x3[0:Dsh], in_=ibc)
                    nc.vector.tensor_copy(out=x3[Dsh:2 * Dsh],
                                          in_=ibmc[:, ::-1])
                    nc.vector.tensor_copy(out=x3[2 * Dsh:3 * Dsh], in_=ibc)
                    dtx96 = s_dt.tile([K * Dsh, Tc], BF16, tag="dtx96")
                    nc.vector.tensor_mul(dtx96, dt96, x3)
                    # bounce dt96 -> [8, 12*Tc] base-0 staging
                    nc.sync.dma_start(out=dtg_dram[par], in_=dt96)
                    g8 = s_g8.tile([DG, K * G, Tc], BF16, tag="g8")
                    nc.sync.dma_start(
                        out=g8,
                        in_=dram_ap(dtg_dram, par * (K * Dsh * Tc),
                                    [[Tc, DG], [DG * Tc, K * G], [1, Tc]]))
                    # B/C replicated loads
                    brep = {}
                    crep = {}
                    for k in (0, 2):
                        bk = s_rep.tile([128, Tc], BF16, tag=f"brep{k}")
                        nc.scalar.dma_start(
                            out=bk, in_=rep_dram(
                                xdbl_full,
                                xoff + (k * XD + R) * L2 + ch * Tc,
                                L2, N, DG, 1, Tc))
                        brep[k] = bk
                        ck = s_rep.tile([128, Tc], BF16, tag=f"crep{k}")
                        nc.scalar.dma_start(
                            out=ck, in_=rep_dram(
                                xdbl_full,
                                xoff + (k * XD + R + N) * L2 + ch * Tc,
                                L2, N, DG, 1, Tc))
                        crep[k] = ck
                    b1 = s_rep.tile([128, Tc], BF16, tag="brep1")
                    nc.scalar.dma_start(
                        out=b1, in_=rep_dram(brc_dram, par * Tc,
                                             2 * Tc, N, DG, 1, Tc))
                    brep[1] = b1
                    c1 = s_rep.tile([128, Tc], BF16, tag="crep1")
                    nc.scalar.dma_start(
                        out=c1, in_=rep_dram(brc_dram, (2 * N + par) * Tc,
                                             2 * Tc, N, DG, 1, Tc))
                    crep[1] = c1

                    y_ps = s_y_ps.tile([2 * Dsh, Tc], F32, tag="yps")
                    # D*x folded into the y02 rows (start of accumulation)
                    for h in range(Tc // HMM):
                        hs = slice(h * HMM, (h + 1) * HMM)
                        nc.tensor.matmul(y_ps[0:Dsh, hs], Ddiag_s,
                                         ibc[:, hs], start=True, stop=False,
                                         skip_group_check=True)

                    for k in range(K):
                        for g in range(G):
                            ci = k * G + g
                            a_t = s_a.tile([128, Tc], BF16, tag="a")
                            for h in range(Tc // HMM):
                                hs = slice(h * HMM, (h + 1) * HMM)
                                ndp = s_nd_ps.tile([128, HMM], F32, tag="ndp")
                                nc.tensor.matmul(ndp, seldd_s,
                                                 g8[:, ci, hs],
                                                 start=True, stop=True,
                                                 skip_group_check=True)
                                nc.scalar.activation(
                                    out=a_t[:, hs], in_=ndp, func=AF.Exp,
                                    scale=A_col[:, ci:ci + 1])
                            dtxr = s_rep.tile([128, Tc], BF16, tag="dxr")
                            dq = nc.sync if ci % 2 == 0 else nc.scalar
                            dq.dma_start(
                                out=dtxr,
                                in_=rep_sbuf(
                                    dtx96[k * Dsh + g * DG:
                                          k * Dsh + (g + 1) * DG, :], N))
                            b_t = s_b.tile([128, Tc], BF16, tag="bt")
                            nc.vector.tensor_mul(b_t, dtxr, brep[k])
                            h_t = s_h.tile([128, Tc], BF16, tag=f"h{ci}")
                            key = (b, ci)
                            init = 0.0 if ch == 0 else \
                                h_prev[key][:, Tc - 1:Tc]
                            nc.vector.tensor_tensor_scan(
                                h_t, a_t, b_t, init, OP.mult, OP.add)
                            h_prev[key] = h_t
                            hc = s_hc.tile([128, Tc], BF16, tag="hc")
                            nc.vector.tensor_mul(hc, h_t, crep[k])
                            ro = 0 if k != 1 else Dsh
                            last = (k == 2 and g == G - 1) if k != 1 \
                                else (g == G - 1)
                            for h in range(Tc // HMM):
                                hs = slice(h * HMM, (h + 1) * HMM)
                                nc.tensor.matmul(
                                    y_ps[ro:ro + Dsh, hs], red_s[:, g, :],
                                    hc[:, hs],
                                    start=(k == 1 and g == 0),
                                    stop=last, skip_group_check=True)
                    yev = s_io.tile([2 * Dsh, Tc], BF16, tag="yev")
                    nc.scalar.copy(out=yev, in_=y_ps)
                    nc.sync.dma_start(
                        out=dram_ap(yev_dram, b * 2 * L2 + ch * Tc,
                                    [[L2, 2], [B * 2 * L2, Dsh], [1, Tc]]),
                        in_=yev)

            phase_scan(0)
            allreduce(xdbl_part[1], xdbl_full[1])
            phase_scan(1)

        # ============ Phase L: LN stats + allreduce + apply + out_proj ======
        half = 2 * B * L
        with tc.tile_pool(name="l_ps", bufs=2, space="PSUM") as l_ps, \
             tc.tile_pool(name="l_one", bufs=1) as l_one, \
             tc.tile_pool(name="l_sq", bufs=3) as l_sq, \
             tc.tile_pool(name="l_stg", bufs=4) as l_stg:
            ydp = l_one.tile([128, L], BF16, tag="ydp")
            MC = 1024
            for b in range(B):
                for jj in range(L2 // MC):
                    yft = l_stg.tile([Dsh, MC], BF16, tag="yft")
                    nc.sync.dma_start(
                        out=yft, in_=yev_dram[:, b, 0, jj * MC:(jj + 1) * MC])
                    y1t = l_stg.tile([Dsh, MC], BF16, tag="y1t")
                    nc.sync.dma_start(
                        out=y1t,
                        in_=yev_dram[:, b, 1, L2 - (jj + 1) * MC:L2 - jj * MC])
                    ym = l_stg.tile([Dsh, MC], BF16, tag="ym")
                    nc.vector.tensor_add(ym, yft, y1t[:, ::-1])
                    tok = slice(jj * MC // 2, (jj + 1) * MC // 2)
                    nc.vector.tensor_copy(
                        out=ydp[(2 * b) * Dsh:(2 * b + 1) * Dsh, tok],
                        in_=ym[:, 0::2])
                    nc.vector.tensor_copy(
                        out=ydp[(2 * b + 1) * Dsh:(2 * b + 2) * Dsh, tok],
                        in_=ym[:, 1::2])
            for j in range(nLch):
                js = slice(j * TcL, (j + 1) * TcL)
                sqp = l_sq.tile([128, TcL], BF16, tag="sqp")
                nc.vector.tensor_mul(sqp, ydp[:, js], ydp[:, js])
                sp = l_ps.tile([4 * B, TcL], F32, tag="sps")
                mm(nc, sp, selst[:, 0, :], ydp[:, js], start=True, stop=False)
                mm(nc, sp, selst[:, 1, :], sqp, start=False, stop=True)
                stg = l_stg.tile([4 * B, TcL], F32, tag="sstg2")
                nc.scalar.copy(out=stg, in_=sp)
                nc.sync.dma_start(
                    out=stats_part.rearrange("a x l -> (a x) l")[:, js], in_=stg)
            allreduce(stats_part, stats_full)
            s1f = l_one.tile([64, SC], F32, tag="s1f")
            s2f = l_one.tile([64, SC], F32, tag="s2f")
            flat = stats_full.rearrange("a x l -> (a x l)")
            nc.sync.dma_start(
                out=s1f, in_=flat[0:half].rearrange("(p c) -> p c", p=64))
            nc.sync.dma_start(
                out=s2f, in_=flat[half:2 * half].rearrange("(p c) -> p c", p=64))
            mu_t = l_one.tile([64, SC], F32, tag="mu_t")
            var_t = l_one.tile([64, SC], F32, tag="var_t")
            musq = l_one.tile([64, SC], F32, tag="musq")
            eps_t = l_one.tile([64, 1], F32, tag="eps_t")
            nc.vector.memset(eps_t, 1e-5)
            nc.vector.tensor_scalar_mul(mu_t, s1f, 1.0 / D)
            nc.vector.tensor_scalar_mul(var_t, s2f, 1.0 / D)
            nc.vector.tensor_mul(musq, mu_t, mu_t)
            nc.vector.tensor_sub(var_t, var_t, musq)
            nc.scalar.activation(out=var_t, in_=var_t, func=AF.Sqrt, bias=eps_t)
            nc.vector.reciprocal(out=s1f, in_=var_t)          # alpha
            nc.vector.tensor_mul(s2f, mu_t, s1f)
            nc.vector.tensor_scalar_mul(s2f, s2f, -1.0)       # beta
            nc.sync.dma_start(
                out=ab_dram.rearrange("a x l -> (a x l)")[0:half].rearrange(
                    "(p c) -> p c", p=64), in_=s1f)
            nc.sync.dma_start(
                out=ab_dram.rearrange("a x l -> (a x l)")[half:2 * half].rearrange(
                    "(p c) -> p c", p=64), in_=s2f)

            # apply LN + gate + out_proj; stream s=0 fully, RS, then s=1, RS
            with tc.tile_pool(name="l_rep", bufs=3) as l_rep, \
                 tc.tile_pool(name="l_t", bufs=3) as l_t, \
                 tc.tile_pool(name="l_fg", bufs=1) as l_fg, \
                 tc.tile_pool(name="o_st", bufs=2) as o_st, \
                 tc.tile_pool(name="o_ps", bufs=2, space="PSUM") as o_ps:
                fgs = []
                for j in range(nLch):
                    js = slice(j * TcL, (j + 1) * TcL)
                    zcp = l_rep.tile([128, TcL], BF16, tag="zcp")
                    nc.sync.dma_start(
                        out=zcp,
                        in_=bass.AP(tensor=zs_dram.tensor,
                                    offset=zs_dram.offset + j * TcL,
                                    ap=[[L, 2 * B], [2 * B * L, Dsh],
                                        [1, TcL]]))
                    zwp = l_t.tile([128, TcL], BF16, tag="zwp")
                    bzp = l_t.tile([128, TcL], BF16, tag="bzp")
                    nc.vector.tensor_scalar_mul(zwp, zcp, lnwb4[:, 0:1])
                    nc.vector.tensor_scalar_mul(bzp, zcp, lnwb4[:, 1:2])
                    arp = l_rep.tile([128, TcL], F32, tag="arp")
                    brp = l_rep.tile([128, TcL], F32, tag="brp")
                    nc.sync.dma_start(
                        out=arp,
                        in_=bass.AP(tensor=ab_dram.tensor,
                                    offset=ab_dram.offset + j * TcL,
                                    ap=[[L, 2 * B], [0, Dsh], [1, TcL]]))
                    nc.scalar.dma_start(
                        out=brp,
                        in_=bass.AP(tensor=ab_dram.tensor,
                                    offset=ab_dram.offset + half + j * TcL,
                                    ap=[[L, 2 * B], [0, Dsh], [1, TcL]]))
                    t1 = l_t.tile([128, TcL], BF16, tag="t1")
                    nc.vector.tensor_mul(t1, ydp[:, js], arp)
                    t2 = l_t.tile([128, TcL], BF16, tag="t2")
                    nc.vector.tensor_add(t2, t1, brp)
                    t3 = l_t.tile([128, TcL], BF16, tag="t3")
                    nc.vector.tensor_mul(t3, t2, zwp)
                    fgp = l_fg.tile([128, TcL], BF16, tag=f"fgp{j}")
                    nc.vector.tensor_add(fgp, t3, bzp)
                    fgs.append(fgp)
                for s in range(2):
                    for j in range(nLch):
                        js = slice(j * TcL, (j + 1) * TcL)
                        fgp = fgs[j]
                        for b in range(B):
                            bs = b * 2 + s
                            qs = slice(bs * Dsh, (bs + 1) * Dsh)
                            if bs * Dsh in (0, 32, 64):
                                lhs = opw4[qs, :]
                                rhs = fgp[qs, :]
                            else:
                                stq = o_st.tile([Dsh, TcL], BF16, tag="stq")
                                nc.sync.dma_start(out=stq, in_=fgp[qs, :])
                                lhs = opw_s[:, s, :]
                                rhs = stq
                            ops = o_ps.tile([C, TcL], F32, tag="ops")
                            mm(nc, ops, lhs, rhs, start=True, stop=True)
                            ost = o_st.tile([C, TcL], F32, tag="ost")
                            nc.scalar.copy(out=ost, in_=ops)
                            nc.sync.dma_start(out=outp_part[s, b, :, js],
                                              in_=ost)
                    # ReduceScatter this stream's half
                    hflat = flat_out // 2
                    hchunk = chunk_o // 2
                    pflat = outp_part.rearrange("s b c l -> (s b c l)")
                    if cfg.fake_cc:
                        nc.sync.dma_start(
                            out=out_chunk[s * hchunk:(s + 1) * hchunk],
                            in_=pflat[s * hflat:s * hflat + hchunk])
                    else:
                        nc.gpsimd.collective_compute(
                            "ReduceScatter", OP.add, replica_groups=groups,
                            ins=[pflat[s * hflat:(s + 1) * hflat].opt()],
                            outs=[outp_rs[s * hchunk:(s + 1) * hchunk].opt()])
                        nc.sync.dma_start(
                            out=out_chunk[s * hchunk:(s + 1) * hchunk],
                            in_=outp_rs[s * hchunk:(s + 1) * hchunk])

    nc.compile()
    return nc


# ---------------------------------------------------------------------------
# public entry point
# ---------------------------------------------------------------------------

_CACHE = {}


def _get_nc(cfg: Cfg):
    if cfg not in _CACHE:
        _CACHE[cfg] = build_nc(cfg)
    return _CACHE[cfg]


def kernel(**inputs):
    cfg = CFG
    nc = _get_nc(cfg)
    in_maps = host_prep(cfg, inputs)
    res = bass_utils.run_bass_kernel_spmd(
        nc, in_maps, core_ids=list(range(cfg.n_cores)))
    return assemble_outputs(cfg, res.results)


def assemble_outputs(cfg, results):
    """Each core returns its ReduceScatter chunks: [ms-chunk | pan-chunk]."""
    B, C, L = cfg.B, cfg.C, cfg.L
    hflat = B * C * L
    hchunk = hflat // cfg.n_cores
    ms_flat = np.zeros(hflat, np.float32)
    pan_flat = np.zeros(hflat, np.float32)
    for r in range(cfg.n_cores):
        ck = np.asarray(results[r]["out_chunk"], np.float32)
        ms_flat[r * hchunk:(r + 1) * hchunk] = ck[0:hchunk]
        pan_flat[r * hchunk:(r + 1) * hchunk] = ck[hchunk:2 * hchunk]
    out_ms = ms_flat.reshape(B, C, cfg.H, cfg.W)
    out_pan = pan_flat.reshape(B, C, cfg.H, cfg.W)
    return (out_ms, out_pan)
